# revision 6
# baseline (speedup 1.0000x reference)
"""Char-level BiLSTM embedder on 8 NeuronCores (Trainium2, Bass/Tile).

Computation: x[B=32,T=128,L=16] char ids -> embed[E=512] -> fwd+bwd LSTM(H=256)
over the L=16 chars of each of the N=B*T=4096 independent words -> final hidden
states concatenated -> y[B,T,2H=512].

Strategy:
  - Data parallel over N: 512 words per core.
  - Embedding lookup + input projection + bias fused on HOST into a single
    [V=128, 4H] LUT per direction:  fused[v,:] = embed[v] @ w_ih.T + b.
    On device the per-step input contribution is a K=128 matmul with a
    one-hot rhs (built on host), accumulated into the same PSUM group as
    the recurrent h matmuls.
  - Everything device-side is feature-major: gates/c/h live as
    [feature-chunk on partitions, words on free dim], so h feeds the next
    step's matmul rhs directly and no transposes are ever needed.
  - Gate order permuted to (i,f,o,g) so activations batch into 3 big ACT ops.
  - fwd and bwd directions interleave per step to hide recurrence latency.
  - Host does the final [2H,n] -> [n,2H] transpose and core concat.
"""

import sys

sys.path.insert(0, "/opt/trn_rl_repo")

import numpy as np
import concourse.bass as bass
import concourse.bacc as bacc
import concourse.mybir as mybir
import concourse.tile as tile
from concourse.bass_utils import run_bass_kernel_spmd

# problem constants (hardcoded per harness contract)
B, T, L = 32, 128, 16
VOCAB, E, H = 128, 512, 256
G4 = 4 * H  # 1024
N_CORES = 8
NW = (B * T) // N_CORES  # 512 words per core

F32 = mybir.dt.float32
# compute dtype for matmul operands / gate activations. bf16 halves PE time
# (fp32 matmuls decompose into 2 passes) and enables DVE 2x modes; the cell
# state c and all PSUM accumulation stay fp32.
DT = mybir.dt.bfloat16

AFT = mybir.ActivationFunctionType


def build_nc():
    nc = bacc.Bacc()

    oh_d = nc.dram_tensor("oh", [L, VOCAB, NW], DT, kind="ExternalInput")
    fused_d = {
        d: nc.dram_tensor(f"fused_{d}", [VOCAB, G4], DT, kind="ExternalInput")
        for d in "fb"
    }
    whh_d = {
        d: nc.dram_tensor(f"whh_{d}", [2, 128, G4], DT, kind="ExternalInput")
        for d in "fb"
    }
    hout_d = nc.dram_tensor("hout", [4, 128, NW], F32, kind="ExternalOutput")

    with tile.TileContext(nc) as tc:
        with (
            tc.tile_pool(name="const", bufs=1) as cpool,
            tc.tile_pool(name="work", bufs=2) as wpool,
            tc.tile_pool(name="state", bufs=2) as spool,
            tc.tile_pool(name="psum", bufs=2, space=bass.MemorySpace.PSUM) as ppool,
        ):
            # --- load constants -------------------------------------------
            fused = {}
            whh = {}
            for d in "fb":
                fused[d] = cpool.tile([128, G4], DT, name=f"fused_{d}_sb", tag=f"fused_{d}")
                nc.sync.dma_start(fused[d][:], fused_d[d][:])
                whh[d] = []
                for k in range(2):
                    w = cpool.tile([128, G4], DT, name=f"whh_{d}{k}_sb", tag=f"whh_{d}{k}")
                    nc.sync.dma_start(w[:], whh_d[d][k])
                    whh[d].append(w)

            # one tile per char position, each with its own DMA (keeps the
            # per-matmul sync-wait count low), loaded in the order the two
            # directions will consume them
            load_order = []
            for t in range(L):
                for tc_ in (t, L - 1 - t):
                    if tc_ not in load_order:
                        load_order.append(tc_)
            oh_tiles = [None] * L
            for t in load_order:
                ot = cpool.tile([128, NW], DT, name=f"oh_{t}", tag=f"oh_{t}")
                nc.sync.dma_start(ot[:], oh_d[t])
                oh_tiles[t] = ot

            out_sb = cpool.tile([128, 4 * NW], F32, name="out_sb", tag="out_sb")

            c_cur = {"f": None, "b": None}
            h_cur = {"f": None, "b": None}

            # --- recurrent steps ------------------------------------------
            # psum_a chunks: i0,i1,f0,f1 (all sigmoid)
            # psum_b chunks: g0,g1,o0,o1 (tanh first so the cell-update
            #   chain can start while the o matmuls still run)
            B_GC = (6, 7, 4, 5)  # psum_b slice jj -> global gate chunk

            def emit_mms(d, t):
                tchar = t if d == "f" else L - 1 - t
                rhs_oh = oh_tiles[tchar][:]
                h_prev = h_cur[d]
                psum_a = ppool.tile([128, 4 * NW], F32, name="psum_a", tag="ps")
                psum_b = ppool.tile([128, 4 * NW], F32, name="psum_b", tag="ps")
                for ps, gcs in ((psum_a, (0, 1, 2, 3)), (psum_b, B_GC)):
                    for jj, gc in enumerate(gcs):
                        sl = ps[:, jj * NW : (jj + 1) * NW]
                        lhs_f = fused[d][:, gc * 128 : (gc + 1) * 128]
                        if h_prev is None:
                            nc.tensor.matmul(sl, lhs_f, rhs_oh, start=True, stop=True)
                        else:
                            nc.tensor.matmul(sl, lhs_f, rhs_oh, start=True, stop=False)
                            for k in range(2):
                                lhs_h = whh[d][k][:, gc * 128 : (gc + 1) * 128]
                                rhs_h = h_prev[:, k * NW : (k + 1) * NW]
                                nc.tensor.matmul(
                                    sl, lhs_h, rhs_h, start=False, stop=(k == 1)
                                )
                return psum_a, psum_b

            def emit_gates(d, psum_a, psum_b):
                sig_if = wpool.tile([128, 4 * NW], DT, name="sig_if", tag=f"sig_if_{d}")
                nc.scalar.activation(sig_if[:], psum_a[:], AFT.Sigmoid)
                tanh_g = wpool.tile([128, 2 * NW], DT, name="tanh_g", tag=f"tanh_g_{d}")
                nc.scalar.activation(tanh_g[:], psum_b[:, 0 : 2 * NW], AFT.Tanh)
                sig_o = wpool.tile([128, 2 * NW], DT, name="sig_o", tag=f"sig_o_{d}")
                nc.scalar.activation(sig_o[:], psum_b[:, 2 * NW : 4 * NW], AFT.Sigmoid)
                return sig_if, tanh_g, sig_o

            def emit_cell(d, t, sig_if, tanh_g):
                # c = sig(f) * c + sig(i) * tanh(g)
                c_prev = c_cur[d]
                c_new = spool.tile([128, 2 * NW], F32, name=f"c_{d}", tag=f"c_{d}")
                if c_prev is None:
                    nc.vector.tensor_mul(c_new[:], sig_if[:, 0 : 2 * NW], tanh_g[:])
                else:
                    m1 = wpool.tile([128, 2 * NW], F32, name="m1", tag=f"m1_{d}")
                    nc.vector.tensor_mul(m1[:], sig_if[:, 2 * NW : 4 * NW], c_prev[:])
                    m2 = wpool.tile([128, 2 * NW], DT, name="m2", tag=f"m2_{d}")
                    nc.vector.tensor_mul(m2[:], sig_if[:, 0 : 2 * NW], tanh_g[:])
                    nc.vector.tensor_add(c_new[:], m1[:], m2[:])
                c_cur[d] = c_new

            def emit_tanh_c(d):
                tanh_c = wpool.tile([128, 2 * NW], DT, name="tanh_c", tag=f"tanh_c_{d}")
                nc.scalar.activation(tanh_c[:], c_cur[d][:], AFT.Tanh)
                return tanh_c

            def emit_h(d, t, sig_o, tanh_c):
                # h = sig(o) * tanh(c) on GpSimd (off the DVE critical path)
                if t == L - 1:
                    off = 0 if d == "f" else 2 * NW
                    nc.gpsimd.tensor_mul(
                        out_sb[:, off : off + 2 * NW], sig_o[:], tanh_c[:]
                    )
                else:
                    h_new = spool.tile([128, 2 * NW], DT, name=f"h_{d}", tag=f"h_{d}")
                    nc.gpsimd.tensor_mul(h_new[:], sig_o[:], tanh_c[:])
                    h_cur[d] = h_new

            for t in range(L):
                pa_f, pb_f = emit_mms("f", t)
                pa_b, pb_b = emit_mms("b", t)
                gates_f = emit_gates("f", pa_f, pb_f)
                emit_cell("f", t, gates_f[0], gates_f[1])
                gates_b = emit_gates("b", pa_b, pb_b)
                tc_f = emit_tanh_c("f")
                emit_cell("b", t, gates_b[0], gates_b[1])
                emit_h("f", t, gates_f[2], tc_f)
                tc_b = emit_tanh_c("b")
                emit_h("b", t, gates_b[2], tc_b)

            for q in range(4):
                nc.sync.dma_start(hout_d[q], out_sb[:, q * NW : (q + 1) * NW])

    nc.compile()
    return nc


_NC_CACHE = None


def _get_nc():
    global _NC_CACHE
    if _NC_CACHE is None:
        _NC_CACHE = build_nc()
    return _NC_CACHE


# gate permutation: torch order (i,f,g,o) -> device order (i,f,o,g)
_PERM = np.concatenate([np.arange(0, 512), np.arange(768, 1024), np.arange(512, 768)])


def _np_dt(dt):
    return mybir.dt.np(dt)


def prepare_in_maps(x, embed_table, w_ih_f, w_hh_f, b_ih_f, b_hh_f,
                    w_ih_b, w_hh_b, b_ih_b, b_hh_b):
    cdt = _np_dt(DT)
    ids = np.asarray(x).reshape(B * T, L).astype(np.int64)

    shared = {}
    for d, w_ih, w_hh, b_ih, b_hh in (
        ("f", w_ih_f, w_hh_f, b_ih_f, b_hh_f),
        ("b", w_ih_b, w_hh_b, b_ih_b, b_hh_b),
    ):
        w_ih = np.asarray(w_ih, np.float32)[_PERM]
        w_hh = np.asarray(w_hh, np.float32)[_PERM]
        b = (np.asarray(b_ih, np.float32) + np.asarray(b_hh, np.float32))[_PERM]
        fused = np.asarray(embed_table, np.float32) @ w_ih.T + b[None, :]
        shared[f"fused_{d}"] = np.ascontiguousarray(fused.astype(cdt))
        shared[f"whh_{d}"] = np.ascontiguousarray(
            w_hh.T.reshape(2, 128, G4).astype(cdt)
        )

    vrange = np.arange(VOCAB)
    in_maps = []
    for c in range(N_CORES):
        ids_c = ids[c * NW : (c + 1) * NW]  # [NW, L]
        oh = (ids_c.T[:, None, :] == vrange[None, :, None]).astype(cdt)  # [L,V,NW]
        m = dict(shared)
        m["oh"] = np.ascontiguousarray(oh)
        in_maps.append(m)
    return in_maps


def assemble_output(results):
    ys = []
    for c in range(N_CORES):
        hout = results[c]["hout"].astype(np.float32)  # [4,128,NW]
        hf = hout[0:2].reshape(2 * 128, NW)  # [H, NW]
        hb = hout[2:4].reshape(2 * 128, NW)
        ys.append(np.concatenate([hf.T, hb.T], axis=1))  # [NW, 2H]
    y = np.concatenate(ys, axis=0)  # [B*T, 2H]
    return y.reshape(B, T, 2 * H)


def run(in_maps, trace=False):
    nc = _get_nc()
    res = run_bass_kernel_spmd(nc, in_maps, core_ids=list(range(N_CORES)), trace=trace)
    return res


def kernel(**inputs) -> np.ndarray:
    in_maps = prepare_in_maps(**inputs)
    res = run(in_maps, trace=False)
    return assemble_output(res.results)


# revision 7
# speedup vs baseline: 1.1311x; 1.1311x over previous
"""Char-level BiLSTM embedder on 8 NeuronCores (Trainium2, Bass/Tile).

Computation: x[B=32,T=128,L=16] char ids -> embed[E=512] -> fwd+bwd LSTM(H=256)
over the L=16 chars of each of the N=B*T=4096 independent words -> final hidden
states concatenated -> y[B,T,2H=512].

Strategy:
  - Data parallel over N: 512 words per core.
  - Embedding lookup + input projection + bias fused on HOST into a single
    [V=128, 4H] LUT per direction:  fused[v,:] = embed[v] @ w_ih.T + b.
    On device the per-step input contribution is a K=128 matmul with a
    one-hot rhs (built on host), accumulated into the same PSUM group as
    the recurrent h matmuls.
  - Everything device-side is feature-major: gates/c/h live as
    [feature-chunk on partitions, words on free dim], so h feeds the next
    step's matmul rhs directly and no transposes are ever needed.
  - Gate order permuted to (i,f,o,g) so activations batch into 3 big ACT ops.
  - fwd and bwd directions interleave per step to hide recurrence latency.
  - Host does the final [2H,n] -> [n,2H] transpose and core concat.
"""

import sys

sys.path.insert(0, "/opt/trn_rl_repo")

import numpy as np
import concourse.bass as bass
import concourse.bacc as bacc
import concourse.mybir as mybir
import concourse.tile as tile
from concourse.bass_utils import run_bass_kernel_spmd

# problem constants (hardcoded per harness contract)
B, T, L = 32, 128, 16
VOCAB, E, H = 128, 512, 256
G4 = 4 * H  # 1024
N_CORES = 8
NW = (B * T) // N_CORES  # 512 words per core

F32 = mybir.dt.float32
# compute dtype for matmul operands / gate activations. bf16 halves PE time
# (fp32 matmuls decompose into 2 passes) and enables DVE 2x modes; the cell
# state c and all PSUM accumulation stay fp32.
DT = mybir.dt.bfloat16

AFT = mybir.ActivationFunctionType


def build_nc():
    nc = bacc.Bacc()

    oh_d = nc.dram_tensor("oh", [L, VOCAB, NW], DT, kind="ExternalInput")
    fused_d = {
        d: nc.dram_tensor(f"fused_{d}", [VOCAB, G4], DT, kind="ExternalInput")
        for d in "fb"
    }
    whh_d = {
        d: nc.dram_tensor(f"whh_{d}", [2, 128, G4], DT, kind="ExternalInput")
        for d in "fb"
    }
    hout_d = nc.dram_tensor("hout", [4, 128, NW], F32, kind="ExternalOutput")

    with tile.TileContext(nc) as tc:
        with (
            tc.tile_pool(name="const", bufs=1) as cpool,
            tc.tile_pool(name="work", bufs=2) as wpool,
            tc.tile_pool(name="state", bufs=2) as spool,
            tc.tile_pool(name="psum", bufs=2, space=bass.MemorySpace.PSUM) as ppool,
        ):
            # --- load constants -------------------------------------------
            fused = {}
            whh = {}
            for d in "fb":
                fused[d] = cpool.tile([128, G4], DT, name=f"fused_{d}_sb", tag=f"fused_{d}")
                nc.sync.dma_start(fused[d][:], fused_d[d][:])
                whh[d] = []
                for k in range(2):
                    w = cpool.tile([128, G4], DT, name=f"whh_{d}{k}_sb", tag=f"whh_{d}{k}")
                    nc.sync.dma_start(w[:], whh_d[d][k])
                    whh[d].append(w)

            # one tile per char position, each with its own DMA (keeps the
            # per-matmul sync-wait count low), loaded in the order the two
            # directions will consume them
            load_order = []
            for t in range(L):
                for tc_ in (t, L - 1 - t):
                    if tc_ not in load_order:
                        load_order.append(tc_)
            oh_tiles = [None] * L
            for t in load_order:
                ot = cpool.tile([128, NW], DT, name=f"oh_{t}", tag=f"oh_{t}")
                nc.sync.dma_start(ot[:], oh_d[t])
                oh_tiles[t] = ot

            out_sb = cpool.tile([128, 4 * NW], F32, name="out_sb", tag="out_sb")

            c_cur = {"f": None, "b": None}
            h_cur = {"f": None, "b": None}

            # --- recurrent steps ------------------------------------------
            # psum_a chunks: i0,i1,f0,f1 (all sigmoid)
            # psum_b chunks: g0,g1,o0,o1 (tanh first so the cell-update
            #   chain can start while the o matmuls still run)
            B_GC = (6, 7, 4, 5)  # psum_b slice jj -> global gate chunk

            def emit_mms(d, t):
                tchar = t if d == "f" else L - 1 - t
                rhs_oh = oh_tiles[tchar][:]
                h_prev = h_cur[d]
                psum_a = ppool.tile([128, 4 * NW], F32, name="psum_a", tag="ps")
                psum_b = ppool.tile([128, 4 * NW], F32, name="psum_b", tag="ps")
                for ps, gcs in ((psum_a, (0, 1, 2, 3)), (psum_b, B_GC)):
                    for jj, gc in enumerate(gcs):
                        sl = ps[:, jj * NW : (jj + 1) * NW]
                        lhs_f = fused[d][:, gc * 128 : (gc + 1) * 128]
                        if h_prev is None:
                            nc.tensor.matmul(sl, lhs_f, rhs_oh, start=True, stop=True)
                        else:
                            nc.tensor.matmul(sl, lhs_f, rhs_oh, start=True, stop=False)
                            for k in range(2):
                                lhs_h = whh[d][k][:, gc * 128 : (gc + 1) * 128]
                                rhs_h = h_prev[:, k * NW : (k + 1) * NW]
                                nc.tensor.matmul(
                                    sl, lhs_h, rhs_h, start=False, stop=(k == 1)
                                )
                return psum_a, psum_b

            def emit_gates(d, psum_a, psum_b):
                sig_if = wpool.tile([128, 4 * NW], DT, name="sig_if", tag=f"sig_if_{d}")
                nc.scalar.activation(sig_if[:], psum_a[:], AFT.Sigmoid)
                tanh_g = wpool.tile([128, 2 * NW], DT, name="tanh_g", tag=f"tanh_g_{d}")
                nc.scalar.activation(tanh_g[:], psum_b[:, 0 : 2 * NW], AFT.Tanh)
                sig_o = wpool.tile([128, 2 * NW], DT, name="sig_o", tag=f"sig_o_{d}")
                nc.scalar.activation(sig_o[:], psum_b[:, 2 * NW : 4 * NW], AFT.Sigmoid)
                return sig_if, tanh_g, sig_o

            def emit_cell(d, t, sig_if, tanh_g):
                # c = sig(f) * c + sig(i) * tanh(g)
                c_prev = c_cur[d]
                c_new = spool.tile([128, 2 * NW], F32, name=f"c_{d}", tag=f"c_{d}")
                if c_prev is None:
                    nc.vector.tensor_mul(c_new[:], sig_if[:, 0 : 2 * NW], tanh_g[:])
                else:
                    m1 = wpool.tile([128, 2 * NW], F32, name="m1", tag=f"m1_{d}")
                    nc.vector.tensor_mul(m1[:], sig_if[:, 2 * NW : 4 * NW], c_prev[:])
                    m2 = wpool.tile([128, 2 * NW], DT, name="m2", tag=f"m2_{d}")
                    nc.vector.tensor_mul(m2[:], sig_if[:, 0 : 2 * NW], tanh_g[:])
                    nc.vector.tensor_add(c_new[:], m1[:], m2[:])
                c_cur[d] = c_new

            def emit_tanh_c(d):
                tanh_c = wpool.tile([128, 2 * NW], DT, name="tanh_c", tag=f"tanh_c_{d}")
                nc.scalar.activation(tanh_c[:], c_cur[d][:], AFT.Tanh)
                return tanh_c

            def emit_h(d, t, sig_o, tanh_c):
                # h = sig(o) * tanh(c)
                if t == L - 1:
                    off = 0 if d == "f" else 2 * NW
                    nc.vector.tensor_mul(
                        out_sb[:, off : off + 2 * NW], sig_o[:], tanh_c[:]
                    )
                else:
                    h_new = spool.tile([128, 2 * NW], DT, name=f"h_{d}", tag=f"h_{d}")
                    nc.vector.tensor_mul(h_new[:], sig_o[:], tanh_c[:])
                    h_cur[d] = h_new

            for t in range(L):
                pa_f, pb_f = emit_mms("f", t)
                pa_b, pb_b = emit_mms("b", t)
                gates_f = emit_gates("f", pa_f, pb_f)
                emit_cell("f", t, gates_f[0], gates_f[1])
                gates_b = emit_gates("b", pa_b, pb_b)
                tc_f = emit_tanh_c("f")
                emit_cell("b", t, gates_b[0], gates_b[1])
                emit_h("f", t, gates_f[2], tc_f)
                tc_b = emit_tanh_c("b")
                emit_h("b", t, gates_b[2], tc_b)

            for q in range(4):
                nc.sync.dma_start(hout_d[q], out_sb[:, q * NW : (q + 1) * NW])

    nc.compile()
    return nc


_NC_CACHE = None


def _get_nc():
    global _NC_CACHE
    if _NC_CACHE is None:
        _NC_CACHE = build_nc()
    return _NC_CACHE


# gate permutation: torch order (i,f,g,o) -> device order (i,f,o,g)
_PERM = np.concatenate([np.arange(0, 512), np.arange(768, 1024), np.arange(512, 768)])


def _np_dt(dt):
    return mybir.dt.np(dt)


def prepare_in_maps(x, embed_table, w_ih_f, w_hh_f, b_ih_f, b_hh_f,
                    w_ih_b, w_hh_b, b_ih_b, b_hh_b):
    cdt = _np_dt(DT)
    ids = np.asarray(x).reshape(B * T, L).astype(np.int64)

    shared = {}
    for d, w_ih, w_hh, b_ih, b_hh in (
        ("f", w_ih_f, w_hh_f, b_ih_f, b_hh_f),
        ("b", w_ih_b, w_hh_b, b_ih_b, b_hh_b),
    ):
        w_ih = np.asarray(w_ih, np.float32)[_PERM]
        w_hh = np.asarray(w_hh, np.float32)[_PERM]
        b = (np.asarray(b_ih, np.float32) + np.asarray(b_hh, np.float32))[_PERM]
        fused = np.asarray(embed_table, np.float32) @ w_ih.T + b[None, :]
        shared[f"fused_{d}"] = np.ascontiguousarray(fused.astype(cdt))
        shared[f"whh_{d}"] = np.ascontiguousarray(
            w_hh.T.reshape(2, 128, G4).astype(cdt)
        )

    vrange = np.arange(VOCAB)
    in_maps = []
    for c in range(N_CORES):
        ids_c = ids[c * NW : (c + 1) * NW]  # [NW, L]
        oh = (ids_c.T[:, None, :] == vrange[None, :, None]).astype(cdt)  # [L,V,NW]
        m = dict(shared)
        m["oh"] = np.ascontiguousarray(oh)
        in_maps.append(m)
    return in_maps


def assemble_output(results):
    ys = []
    for c in range(N_CORES):
        hout = results[c]["hout"].astype(np.float32)  # [4,128,NW]
        hf = hout[0:2].reshape(2 * 128, NW)  # [H, NW]
        hb = hout[2:4].reshape(2 * 128, NW)
        ys.append(np.concatenate([hf.T, hb.T], axis=1))  # [NW, 2H]
    y = np.concatenate(ys, axis=0)  # [B*T, 2H]
    return y.reshape(B, T, 2 * H)


def run(in_maps, trace=False):
    nc = _get_nc()
    res = run_bass_kernel_spmd(nc, in_maps, core_ids=list(range(N_CORES)), trace=trace)
    return res


def kernel(**inputs) -> np.ndarray:
    in_maps = prepare_in_maps(**inputs)
    res = run(in_maps, trace=False)
    return assemble_output(res.results)


# revision 10
# speedup vs baseline: 1.1391x; 1.0071x over previous
"""Char-level BiLSTM embedder on 8 NeuronCores (Trainium2, Bass/Tile).

Computation: x[B=32,T=128,L=16] char ids -> embed[E=512] -> fwd+bwd LSTM(H=256)
over the L=16 chars of each of the N=B*T=4096 independent words -> final hidden
states concatenated -> y[B,T,2H=512].

Strategy:
  - Data parallel over N: 512 words per core.
  - Embedding lookup + input projection + bias fused on HOST into a single
    [V=128, 4H] LUT per direction:  fused[v,:] = embed[v] @ w_ih.T + b.
    On device the per-step input contribution is a K=128 matmul with a
    one-hot rhs (built on host), accumulated into the same PSUM group as
    the recurrent h matmuls.
  - Everything device-side is feature-major: gates/c/h live as
    [feature-chunk on partitions, words on free dim], so h feeds the next
    step's matmul rhs directly and no transposes are ever needed.
  - Gate order permuted to (i,f,o,g) so activations batch into 3 big ACT ops.
  - fwd and bwd directions interleave per step to hide recurrence latency.
  - Host does the final [2H,n] -> [n,2H] transpose and core concat.
"""

import sys

sys.path.insert(0, "/opt/trn_rl_repo")

import numpy as np
import concourse.bass as bass
import concourse.bacc as bacc
import concourse.mybir as mybir
import concourse.tile as tile
from concourse.bass_utils import run_bass_kernel_spmd

# problem constants (hardcoded per harness contract)
B, T, L = 32, 128, 16
VOCAB, E, H = 128, 512, 256
G4 = 4 * H  # 1024
N_CORES = 8
NW = (B * T) // N_CORES  # 512 words per core

F32 = mybir.dt.float32
# compute dtype for matmul operands / gate activations. bf16 halves PE time
# (fp32 matmuls decompose into 2 passes) and enables DVE 2x modes; the cell
# state c and all PSUM accumulation stay fp32.
DT = mybir.dt.bfloat16

AFT = mybir.ActivationFunctionType


def build_nc():
    nc = bacc.Bacc()

    # onehots in 4 groups of 4 char positions; weights packed per direction
    # as [fused, whhT_k0, whhT_k1] so the whole input side is 6 DMAs.
    oh_d = nc.dram_tensor("oh", [4, 4, VOCAB, NW], DT, kind="ExternalInput")
    wts_d = {
        d: nc.dram_tensor(f"wts_{d}", [3, 128, G4], DT, kind="ExternalInput")
        for d in "fb"
    }
    hout_d = nc.dram_tensor("hout", [128, 4 * NW], F32, kind="ExternalOutput")

    with tile.TileContext(nc) as tc:
        with (
            tc.tile_pool(name="const", bufs=1) as cpool,
            tc.tile_pool(name="work", bufs=2) as wpool,
            tc.tile_pool(name="state", bufs=2) as spool,
            tc.tile_pool(name="psum", bufs=2, space=bass.MemorySpace.PSUM) as ppool,
        ):
            # --- load constants (6 input DMAs total) ----------------------
            wts = {}
            oh_grp = [None] * 4
            for d, og in (("f", 0), ("b", 3)):
                w = cpool.tile([128, 3 * G4], DT, name=f"wts_{d}_sb", tag=f"wts_{d}")
                nc.sync.dma_start(
                    w[:].rearrange("p (k g) -> p k g", k=3),
                    wts_d[d].rearrange("k p g -> p k g"),
                )
                wts[d] = w
                ot = cpool.tile([128, 4 * NW], DT, name=f"oh_g{og}", tag=f"oh_g{og}")
                nc.sync.dma_start(
                    ot[:].rearrange("p (t n) -> p t n", t=4),
                    oh_d[og].rearrange("t p n -> p t n"),
                )
                oh_grp[og] = ot
            for og in (1, 2):
                ot = cpool.tile([128, 4 * NW], DT, name=f"oh_g{og}", tag=f"oh_g{og}")
                nc.sync.dma_start(
                    ot[:].rearrange("p (t n) -> p t n", t=4),
                    oh_d[og].rearrange("t p n -> p t n"),
                )
                oh_grp[og] = ot

            fused = {d: wts[d][:, 0:G4] for d in "fb"}
            whh = {d: [wts[d][:, G4 : 2 * G4], wts[d][:, 2 * G4 : 3 * G4]] for d in "fb"}

            def oh_rhs(t):
                return oh_grp[t // 4][:, (t % 4) * NW : (t % 4 + 1) * NW]

            out_sb = cpool.tile([128, 4 * NW], F32, name="out_sb", tag="out_sb")

            c_cur = {"f": None, "b": None}
            h_cur = {"f": None, "b": None}

            # --- recurrent steps ------------------------------------------
            # psum_a chunks: i0,i1,f0,f1 (all sigmoid)
            # psum_b chunks: g0,g1,o0,o1 (tanh first so the cell-update
            #   chain can start while the o matmuls still run)
            B_GC = (6, 7, 4, 5)  # psum_b slice jj -> global gate chunk

            def emit_mms(d, t):
                tchar = t if d == "f" else L - 1 - t
                rhs_oh = oh_rhs(tchar)
                h_prev = h_cur[d]
                psum_a = ppool.tile([128, 4 * NW], F32, name="psum_a", tag="ps")
                psum_b = ppool.tile([128, 4 * NW], F32, name="psum_b", tag="ps")
                for ps, gcs in ((psum_a, (0, 1, 2, 3)), (psum_b, B_GC)):
                    for jj, gc in enumerate(gcs):
                        sl = ps[:, jj * NW : (jj + 1) * NW]
                        lhs_f = fused[d][:, gc * 128 : (gc + 1) * 128]
                        if h_prev is None:
                            nc.tensor.matmul(sl, lhs_f, rhs_oh, start=True, stop=True)
                        else:
                            nc.tensor.matmul(sl, lhs_f, rhs_oh, start=True, stop=False)
                            for k in range(2):
                                lhs_h = whh[d][k][:, gc * 128 : (gc + 1) * 128]
                                rhs_h = h_prev[:, k * NW : (k + 1) * NW]
                                nc.tensor.matmul(
                                    sl, lhs_h, rhs_h, start=False, stop=(k == 1)
                                )
                return psum_a, psum_b

            def emit_gates(d, psum_a, psum_b):
                sig_if = wpool.tile([128, 4 * NW], DT, name="sig_if", tag=f"sig_if_{d}")
                nc.scalar.activation(sig_if[:], psum_a[:], AFT.Sigmoid)
                tanh_g = wpool.tile([128, 2 * NW], DT, name="tanh_g", tag=f"tanh_g_{d}")
                nc.scalar.activation(tanh_g[:], psum_b[:, 0 : 2 * NW], AFT.Tanh)
                sig_o = wpool.tile([128, 2 * NW], DT, name="sig_o", tag=f"sig_o_{d}")
                nc.scalar.activation(sig_o[:], psum_b[:, 2 * NW : 4 * NW], AFT.Sigmoid)
                return sig_if, tanh_g, sig_o

            def emit_cell(d, t, sig_if, tanh_g):
                # c = sig(f) * c + sig(i) * tanh(g)
                c_prev = c_cur[d]
                c_new = spool.tile([128, 2 * NW], F32, name=f"c_{d}", tag=f"c_{d}")
                if c_prev is None:
                    nc.vector.tensor_mul(c_new[:], sig_if[:, 0 : 2 * NW], tanh_g[:])
                else:
                    m1 = wpool.tile([128, 2 * NW], F32, name="m1", tag=f"m1_{d}")
                    nc.vector.tensor_mul(m1[:], sig_if[:, 2 * NW : 4 * NW], c_prev[:])
                    m2 = wpool.tile([128, 2 * NW], DT, name="m2", tag=f"m2_{d}")
                    nc.vector.tensor_mul(m2[:], sig_if[:, 0 : 2 * NW], tanh_g[:])
                    nc.vector.tensor_add(c_new[:], m1[:], m2[:])
                c_cur[d] = c_new

            def emit_tanh_c(d):
                tanh_c = wpool.tile([128, 2 * NW], DT, name="tanh_c", tag=f"tanh_c_{d}")
                nc.scalar.activation(tanh_c[:], c_cur[d][:], AFT.Tanh)
                return tanh_c

            def emit_h(d, t, sig_o, tanh_c):
                # h = sig(o) * tanh(c)
                if t == L - 1:
                    off = 0 if d == "f" else 2 * NW
                    nc.vector.tensor_mul(
                        out_sb[:, off : off + 2 * NW], sig_o[:], tanh_c[:]
                    )
                else:
                    h_new = spool.tile([128, 2 * NW], DT, name=f"h_{d}", tag=f"h_{d}")
                    nc.vector.tensor_mul(h_new[:], sig_o[:], tanh_c[:])
                    h_cur[d] = h_new

            for t in range(L):
                pa_f, pb_f = emit_mms("f", t)
                pa_b, pb_b = emit_mms("b", t)
                gates_f = emit_gates("f", pa_f, pb_f)
                emit_cell("f", t, gates_f[0], gates_f[1])
                gates_b = emit_gates("b", pa_b, pb_b)
                tc_f = emit_tanh_c("f")
                emit_cell("b", t, gates_b[0], gates_b[1])
                emit_h("f", t, gates_f[2], tc_f)
                tc_b = emit_tanh_c("b")
                emit_h("b", t, gates_b[2], tc_b)

            nc.sync.dma_start(hout_d[:, 0 : 2 * NW], out_sb[:, 0 : 2 * NW])
            nc.sync.dma_start(hout_d[:, 2 * NW : 4 * NW], out_sb[:, 2 * NW : 4 * NW])

    nc.compile()
    return nc


_NC_CACHE = None


def _get_nc():
    global _NC_CACHE
    if _NC_CACHE is None:
        _NC_CACHE = build_nc()
    return _NC_CACHE


# gate permutation: torch order (i,f,g,o) -> device order (i,f,o,g)
_PERM = np.concatenate([np.arange(0, 512), np.arange(768, 1024), np.arange(512, 768)])


def _np_dt(dt):
    return mybir.dt.np(dt)


def prepare_in_maps(x, embed_table, w_ih_f, w_hh_f, b_ih_f, b_hh_f,
                    w_ih_b, w_hh_b, b_ih_b, b_hh_b):
    cdt = _np_dt(DT)
    ids = np.asarray(x).reshape(B * T, L).astype(np.int64)

    shared = {}
    for d, w_ih, w_hh, b_ih, b_hh in (
        ("f", w_ih_f, w_hh_f, b_ih_f, b_hh_f),
        ("b", w_ih_b, w_hh_b, b_ih_b, b_hh_b),
    ):
        w_ih = np.asarray(w_ih, np.float32)[_PERM]
        w_hh = np.asarray(w_hh, np.float32)[_PERM]
        b = (np.asarray(b_ih, np.float32) + np.asarray(b_hh, np.float32))[_PERM]
        fused = np.asarray(embed_table, np.float32) @ w_ih.T + b[None, :]
        wts = np.concatenate(
            [fused.reshape(1, 128, G4), w_hh.T.reshape(2, 128, G4)], axis=0
        )
        shared[f"wts_{d}"] = np.ascontiguousarray(wts.astype(cdt))

    vrange = np.arange(VOCAB)
    in_maps = []
    for c in range(N_CORES):
        ids_c = ids[c * NW : (c + 1) * NW]  # [NW, L]
        oh = (ids_c.T[:, None, :] == vrange[None, :, None]).astype(cdt)  # [L,V,NW]
        m = dict(shared)
        m["oh"] = np.ascontiguousarray(oh.reshape(4, 4, VOCAB, NW))
        in_maps.append(m)
    return in_maps


def assemble_output(results):
    ys = []
    for c in range(N_CORES):
        hout = results[c]["hout"].astype(np.float32)  # [128, 4*NW]
        hf = np.concatenate([hout[:, 0:NW], hout[:, NW : 2 * NW]], axis=0)  # [H,NW]
        hb = np.concatenate([hout[:, 2 * NW : 3 * NW], hout[:, 3 * NW : 4 * NW]], axis=0)
        ys.append(np.concatenate([hf.T, hb.T], axis=1))  # [NW, 2H]
    y = np.concatenate(ys, axis=0)  # [B*T, 2H]
    return y.reshape(B, T, 2 * H)


def run(in_maps, trace=False):
    nc = _get_nc()
    res = run_bass_kernel_spmd(nc, in_maps, core_ids=list(range(N_CORES)), trace=trace)
    return res


def kernel(**inputs) -> np.ndarray:
    in_maps = prepare_in_maps(**inputs)
    res = run(in_maps, trace=False)
    return assemble_output(res.results)


# revision 11
# speedup vs baseline: 1.1915x; 1.0459x over previous
"""Char-level BiLSTM embedder on 8 NeuronCores (Trainium2, Bass/Tile).

Computation: x[B=32,T=128,L=16] char ids -> embed[E=512] -> fwd+bwd LSTM(H=256)
over the L=16 chars of each of the N=B*T=4096 independent words -> final hidden
states concatenated -> y[B,T,2H=512].

Strategy:
  - Data parallel over N: 512 words per core.
  - Embedding lookup + input projection + bias fused on HOST into a single
    [V=128, 4H] LUT per direction:  fused[v,:] = embed[v] @ w_ih.T + b.
    On device the per-step input contribution is a K=128 matmul with a
    one-hot rhs (built on host), accumulated into the same PSUM group as
    the recurrent h matmuls.
  - Everything device-side is feature-major: gates/c/h live as
    [feature-chunk on partitions, words on free dim], so h feeds the next
    step's matmul rhs directly and no transposes are ever needed.
  - Gate order permuted to (i,f,o,g) so activations batch into 3 big ACT ops.
  - fwd and bwd directions interleave per step to hide recurrence latency.
  - Host does the final [2H,n] -> [n,2H] transpose and core concat.
"""

import sys

sys.path.insert(0, "/opt/trn_rl_repo")

import numpy as np
import concourse.bass as bass
import concourse.bacc as bacc
import concourse.mybir as mybir
import concourse.tile as tile
from concourse.bass_utils import run_bass_kernel_spmd

# problem constants (hardcoded per harness contract)
B, T, L = 32, 128, 16
VOCAB, E, H = 128, 512, 256
G4 = 4 * H  # 1024
N_CORES = 8
NW = (B * T) // N_CORES  # 512 words per core

F32 = mybir.dt.float32
# compute dtype for matmul operands / gate activations. bf16 halves PE time
# (fp32 matmuls decompose into 2 passes) and enables DVE 2x modes; the cell
# state c and all PSUM accumulation stay fp32.
DT = mybir.dt.bfloat16

AFT = mybir.ActivationFunctionType


def build_nc():
    nc = bacc.Bacc()

    # onehots in 4 groups of 4 char positions; weights packed per direction
    # as [fused, whhT_k0, whhT_k1] so the whole input side is 6 DMAs.
    oh_d = nc.dram_tensor("oh", [4, 4, VOCAB, NW], DT, kind="ExternalInput")
    wts_d = {
        d: nc.dram_tensor(f"wts_{d}", [3, 128, G4], DT, kind="ExternalInput")
        for d in "fb"
    }
    hout_d = nc.dram_tensor("hout", [128, 4 * NW], F32, kind="ExternalOutput")

    with tile.TileContext(nc) as tc:
        with (
            tc.tile_pool(name="const", bufs=1) as cpool,
            tc.tile_pool(name="work", bufs=2) as wpool,
            tc.tile_pool(name="state", bufs=2) as spool,
            tc.tile_pool(name="psum", bufs=2, space=bass.MemorySpace.PSUM) as ppool,
        ):
            # --- load constants (6 input DMAs total) ----------------------
            wts = {}
            oh_grp = [None] * 4
            for d, og in (("f", 0), ("b", 3)):
                w = cpool.tile([128, 3 * G4], DT, name=f"wts_{d}_sb", tag=f"wts_{d}")
                nc.sync.dma_start(
                    w[:].rearrange("p (k g) -> p k g", k=3),
                    wts_d[d].rearrange("k p g -> p k g"),
                )
                wts[d] = w
                ot = cpool.tile([128, 4 * NW], DT, name=f"oh_g{og}", tag=f"oh_g{og}")
                nc.sync.dma_start(
                    ot[:].rearrange("p (t n) -> p t n", t=4),
                    oh_d[og].rearrange("t p n -> p t n"),
                )
                oh_grp[og] = ot
            for og in (1, 2):
                ot = cpool.tile([128, 4 * NW], DT, name=f"oh_g{og}", tag=f"oh_g{og}")
                nc.sync.dma_start(
                    ot[:].rearrange("p (t n) -> p t n", t=4),
                    oh_d[og].rearrange("t p n -> p t n"),
                )
                oh_grp[og] = ot

            fused = {d: wts[d][:, 0:G4] for d in "fb"}
            whh = {d: [wts[d][:, G4 : 2 * G4], wts[d][:, 2 * G4 : 3 * G4]] for d in "fb"}

            def oh_rhs(t):
                return oh_grp[t // 4][:, (t % 4) * NW : (t % 4 + 1) * NW]

            out_sb = cpool.tile([128, 4 * NW], F32, name="out_sb", tag="out_sb")

            c_cur = {"f": None, "b": None}
            h_cur = {"f": None, "b": None}

            # --- recurrent steps ------------------------------------------
            # psum_a chunks: i0,i1,f0,f1 (all sigmoid)
            # psum_b chunks: g0,g1,o0,o1 (tanh first so the cell-update
            #   chain can start while the o matmuls still run)
            B_GC = (6, 7, 4, 5)  # psum_b slice jj -> global gate chunk

            def emit_mms(d, t):
                tchar = t if d == "f" else L - 1 - t
                rhs_oh = oh_rhs(tchar)
                h_prev = h_cur[d]
                psum_a = ppool.tile([128, 4 * NW], F32, name="psum_a", tag="ps")
                psum_b = ppool.tile([128, 4 * NW], F32, name="psum_b", tag="ps")
                # all LUT (one-hot) matmuls first: they depend only on
                # constants, so the PE can run them while h is still being
                # computed; the h matmuls follow.
                for ps, gcs in ((psum_a, (0, 1, 2, 3)), (psum_b, B_GC)):
                    for jj, gc in enumerate(gcs):
                        sl = ps[:, jj * NW : (jj + 1) * NW]
                        lhs_f = fused[d][:, gc * 128 : (gc + 1) * 128]
                        nc.tensor.matmul(
                            sl, lhs_f, rhs_oh, start=True, stop=h_prev is None
                        )
                if h_prev is not None:
                    for ps, gcs in ((psum_a, (0, 1, 2, 3)), (psum_b, B_GC)):
                        for jj, gc in enumerate(gcs):
                            sl = ps[:, jj * NW : (jj + 1) * NW]
                            for k in range(2):
                                lhs_h = whh[d][k][:, gc * 128 : (gc + 1) * 128]
                                rhs_h = h_prev[:, k * NW : (k + 1) * NW]
                                nc.tensor.matmul(
                                    sl, lhs_h, rhs_h, start=False, stop=(k == 1)
                                )
                return psum_a, psum_b

            def emit_gates(d, psum_a, psum_b):
                sig_if = wpool.tile([128, 4 * NW], DT, name="sig_if", tag=f"sig_if_{d}")
                nc.scalar.activation(sig_if[:], psum_a[:], AFT.Sigmoid)
                tanh_g = wpool.tile([128, 2 * NW], DT, name="tanh_g", tag=f"tanh_g_{d}")
                nc.scalar.activation(tanh_g[:], psum_b[:, 0 : 2 * NW], AFT.Tanh)
                sig_o = wpool.tile([128, 2 * NW], DT, name="sig_o", tag=f"sig_o_{d}")
                nc.scalar.activation(sig_o[:], psum_b[:, 2 * NW : 4 * NW], AFT.Sigmoid)
                return sig_if, tanh_g, sig_o

            def emit_cell(d, t, sig_if, tanh_g):
                # c = sig(f) * c + sig(i) * tanh(g)
                c_prev = c_cur[d]
                c_new = spool.tile([128, 2 * NW], DT, name=f"c_{d}", tag=f"c_{d}")
                if c_prev is None:
                    nc.vector.tensor_mul(c_new[:], sig_if[:, 0 : 2 * NW], tanh_g[:])
                else:
                    m1 = wpool.tile([128, 2 * NW], DT, name="m1", tag=f"m1_{d}")
                    nc.vector.tensor_mul(m1[:], sig_if[:, 2 * NW : 4 * NW], c_prev[:])
                    m2 = wpool.tile([128, 2 * NW], DT, name="m2", tag=f"m2_{d}")
                    nc.vector.tensor_mul(m2[:], sig_if[:, 0 : 2 * NW], tanh_g[:])
                    nc.vector.tensor_add(c_new[:], m1[:], m2[:])
                c_cur[d] = c_new

            def emit_tanh_c(d):
                tanh_c = wpool.tile([128, 2 * NW], DT, name="tanh_c", tag=f"tanh_c_{d}")
                nc.scalar.activation(tanh_c[:], c_cur[d][:], AFT.Tanh)
                return tanh_c

            def emit_h(d, t, sig_o, tanh_c):
                # h = sig(o) * tanh(c)
                if t == L - 1:
                    off = 0 if d == "f" else 2 * NW
                    nc.vector.tensor_mul(
                        out_sb[:, off : off + 2 * NW], sig_o[:], tanh_c[:]
                    )
                else:
                    h_new = spool.tile([128, 2 * NW], DT, name=f"h_{d}", tag=f"h_{d}")
                    nc.vector.tensor_mul(h_new[:], sig_o[:], tanh_c[:])
                    h_cur[d] = h_new

            for t in range(L):
                pa_f, pb_f = emit_mms("f", t)
                pa_b, pb_b = emit_mms("b", t)
                gates_f = emit_gates("f", pa_f, pb_f)
                emit_cell("f", t, gates_f[0], gates_f[1])
                gates_b = emit_gates("b", pa_b, pb_b)
                tc_f = emit_tanh_c("f")
                emit_cell("b", t, gates_b[0], gates_b[1])
                emit_h("f", t, gates_f[2], tc_f)
                tc_b = emit_tanh_c("b")
                emit_h("b", t, gates_b[2], tc_b)

            nc.sync.dma_start(hout_d[:, 0 : 2 * NW], out_sb[:, 0 : 2 * NW])
            nc.sync.dma_start(hout_d[:, 2 * NW : 4 * NW], out_sb[:, 2 * NW : 4 * NW])

    nc.compile()
    return nc


_NC_CACHE = None


def _get_nc():
    global _NC_CACHE
    if _NC_CACHE is None:
        _NC_CACHE = build_nc()
    return _NC_CACHE


# gate permutation: torch order (i,f,g,o) -> device order (i,f,o,g)
_PERM = np.concatenate([np.arange(0, 512), np.arange(768, 1024), np.arange(512, 768)])


def _np_dt(dt):
    return mybir.dt.np(dt)


def prepare_in_maps(x, embed_table, w_ih_f, w_hh_f, b_ih_f, b_hh_f,
                    w_ih_b, w_hh_b, b_ih_b, b_hh_b):
    cdt = _np_dt(DT)
    ids = np.asarray(x).reshape(B * T, L).astype(np.int64)

    shared = {}
    for d, w_ih, w_hh, b_ih, b_hh in (
        ("f", w_ih_f, w_hh_f, b_ih_f, b_hh_f),
        ("b", w_ih_b, w_hh_b, b_ih_b, b_hh_b),
    ):
        w_ih = np.asarray(w_ih, np.float32)[_PERM]
        w_hh = np.asarray(w_hh, np.float32)[_PERM]
        b = (np.asarray(b_ih, np.float32) + np.asarray(b_hh, np.float32))[_PERM]
        fused = np.asarray(embed_table, np.float32) @ w_ih.T + b[None, :]
        wts = np.concatenate(
            [fused.reshape(1, 128, G4), w_hh.T.reshape(2, 128, G4)], axis=0
        )
        shared[f"wts_{d}"] = np.ascontiguousarray(wts.astype(cdt))

    vrange = np.arange(VOCAB)
    in_maps = []
    for c in range(N_CORES):
        ids_c = ids[c * NW : (c + 1) * NW]  # [NW, L]
        oh = (ids_c.T[:, None, :] == vrange[None, :, None]).astype(cdt)  # [L,V,NW]
        m = dict(shared)
        m["oh"] = np.ascontiguousarray(oh.reshape(4, 4, VOCAB, NW))
        in_maps.append(m)
    return in_maps


def assemble_output(results):
    ys = []
    for c in range(N_CORES):
        hout = results[c]["hout"].astype(np.float32)  # [128, 4*NW]
        hf = np.concatenate([hout[:, 0:NW], hout[:, NW : 2 * NW]], axis=0)  # [H,NW]
        hb = np.concatenate([hout[:, 2 * NW : 3 * NW], hout[:, 3 * NW : 4 * NW]], axis=0)
        ys.append(np.concatenate([hf.T, hb.T], axis=1))  # [NW, 2H]
    y = np.concatenate(ys, axis=0)  # [B*T, 2H]
    return y.reshape(B, T, 2 * H)


def run(in_maps, trace=False):
    nc = _get_nc()
    res = run_bass_kernel_spmd(nc, in_maps, core_ids=list(range(N_CORES)), trace=trace)
    return res


def kernel(**inputs) -> np.ndarray:
    in_maps = prepare_in_maps(**inputs)
    res = run(in_maps, trace=False)
    return assemble_output(res.results)


# revision 13
# speedup vs baseline: 1.2006x; 1.0077x over previous
"""Char-level BiLSTM embedder on 8 NeuronCores (Trainium2, Bass/Tile).

Computation: x[B=32,T=128,L=16] char ids -> embed[E=512] -> fwd+bwd LSTM(H=256)
over the L=16 chars of each of the N=B*T=4096 independent words -> final hidden
states concatenated -> y[B,T,2H=512].

Strategy:
  - Data parallel over N: 512 words per core.
  - Embedding lookup + input projection + bias fused on HOST into a single
    [V=128, 4H] LUT per direction:  fused[v,:] = embed[v] @ w_ih.T + b.
    On device the per-step input contribution is a K=128 matmul with a
    one-hot rhs (built on host), accumulated into the same PSUM group as
    the recurrent h matmuls.
  - Everything device-side is feature-major: gates/c/h live as
    [feature-chunk on partitions, words on free dim], so h feeds the next
    step's matmul rhs directly and no transposes are ever needed.
  - Gate order permuted to (i,f,o,g) so activations batch into 3 big ACT ops.
  - fwd and bwd directions interleave per step to hide recurrence latency.
  - Host does the final [2H,n] -> [n,2H] transpose and core concat.
"""

import sys

sys.path.insert(0, "/opt/trn_rl_repo")

import numpy as np
import concourse.bass as bass
import concourse.bacc as bacc
import concourse.mybir as mybir
import concourse.tile as tile
from concourse.bass_utils import run_bass_kernel_spmd

# problem constants (hardcoded per harness contract)
B, T, L = 32, 128, 16
VOCAB, E, H = 128, 512, 256
G4 = 4 * H  # 1024
N_CORES = 8
NW = (B * T) // N_CORES  # 512 words per core

F32 = mybir.dt.float32
# compute dtype for matmul operands / gate activations. bf16 halves PE time
# (fp32 matmuls decompose into 2 passes) and enables DVE 2x modes; the cell
# state c and all PSUM accumulation stay fp32.
DT = mybir.dt.bfloat16

AFT = mybir.ActivationFunctionType


def build_nc():
    nc = bacc.Bacc()

    # onehots: t=0 and t=15 land first (tiny DMAs) so both directions can
    # start immediately; the middle steps come as two bulk DMAs.
    oh_d = nc.dram_tensor("oh", [L, VOCAB, NW], DT, kind="ExternalInput")
    fused_dd = {
        d: nc.dram_tensor(f"fused_{d}", [VOCAB, G4], DT, kind="ExternalInput")
        for d in "fb"
    }
    whh_dd = {
        d: nc.dram_tensor(f"whh_{d}", [2, 128, G4], DT, kind="ExternalInput")
        for d in "fb"
    }
    hout_d = nc.dram_tensor("hout", [128, 4 * NW], DT, kind="ExternalOutput")

    with tile.TileContext(nc) as tc:
        with (
            tc.tile_pool(name="const", bufs=1) as cpool,
            tc.tile_pool(name="work", bufs=2) as wpool,
            tc.tile_pool(name="state", bufs=2) as spool,
            tc.tile_pool(name="psum", bufs=2, space=bass.MemorySpace.PSUM) as ppool,
        ):
            # --- load constants -------------------------------------------
            fused = {}
            whh = {}
            oh_ends = {}
            for d, te in (("f", 0), ("b", L - 1)):
                fu = cpool.tile([128, G4], DT, name=f"fused_{d}_sb", tag=f"fused_{d}")
                nc.sync.dma_start(fu[:], fused_dd[d][:])
                fused[d] = fu
                ot = cpool.tile([128, NW], DT, name=f"oh_e{te}", tag=f"oh_e{te}")
                nc.sync.dma_start(ot[:], oh_d[te])
                oh_ends[te] = ot
            for d in "fb":
                w = cpool.tile([128, 2 * G4], DT, name=f"whh_{d}_sb", tag=f"whh_{d}")
                nc.sync.dma_start(
                    w[:].rearrange("p (k g) -> p k g", k=2),
                    whh_dd[d].rearrange("k p g -> p k g"),
                )
                whh[d] = [w[:, 0:G4], w[:, G4 : 2 * G4]]
            oh_mid = {}
            for lo, hi in ((1, 8), (8, 15)):
                om = cpool.tile([128, 7 * NW], DT, name=f"oh_m{lo}", tag=f"oh_m{lo}")
                nc.sync.dma_start(
                    om[:].rearrange("p (t n) -> p t n", t=7),
                    oh_d[lo:hi].rearrange("t p n -> p t n"),
                )
                oh_mid[lo] = om

            def oh_rhs(t):
                if t in oh_ends:
                    return oh_ends[t][:]
                lo = 1 if t < 8 else 8
                return oh_mid[lo][:, (t - lo) * NW : (t - lo + 1) * NW]

            out_sb = cpool.tile([128, 4 * NW], DT, name="out_sb", tag="out_sb")

            c_cur = {"f": None, "b": None}
            h_cur = {"f": None, "b": None}

            # --- recurrent steps ------------------------------------------
            # psum_a chunks: i0,i1,f0,f1 (all sigmoid)
            # psum_b chunks: g0,g1,o0,o1 (tanh first so the cell-update
            #   chain can start while the o matmuls still run)
            B_GC = (6, 7, 4, 5)  # psum_b slice jj -> global gate chunk

            def emit_mms(d, t):
                tchar = t if d == "f" else L - 1 - t
                rhs_oh = oh_rhs(tchar)
                h_prev = h_cur[d]
                psum_a = ppool.tile([128, 4 * NW], F32, name="psum_a", tag="ps")
                psum_b = ppool.tile([128, 4 * NW], F32, name="psum_b", tag="ps")
                # all LUT (one-hot) matmuls first: they depend only on
                # constants, so the PE can run them while h is still being
                # computed; the h matmuls follow.
                for ps, gcs in ((psum_a, (0, 1, 2, 3)), (psum_b, B_GC)):
                    for jj, gc in enumerate(gcs):
                        sl = ps[:, jj * NW : (jj + 1) * NW]
                        lhs_f = fused[d][:, gc * 128 : (gc + 1) * 128]
                        nc.tensor.matmul(
                            sl, lhs_f, rhs_oh, start=True, stop=h_prev is None
                        )
                if h_prev is not None:
                    for ps, gcs in ((psum_a, (0, 1, 2, 3)), (psum_b, B_GC)):
                        for jj, gc in enumerate(gcs):
                            sl = ps[:, jj * NW : (jj + 1) * NW]
                            for k in range(2):
                                lhs_h = whh[d][k][:, gc * 128 : (gc + 1) * 128]
                                rhs_h = h_prev[:, k * NW : (k + 1) * NW]
                                nc.tensor.matmul(
                                    sl, lhs_h, rhs_h, start=False, stop=(k == 1)
                                )
                return psum_a, psum_b

            def emit_gates_ifg(d, psum_a, psum_b):
                sig_if = wpool.tile([128, 4 * NW], DT, name="sig_if", tag=f"sig_if_{d}")
                nc.scalar.activation(sig_if[:], psum_a[:], AFT.Sigmoid)
                tanh_g = wpool.tile([128, 2 * NW], DT, name="tanh_g", tag=f"tanh_g_{d}")
                nc.scalar.activation(tanh_g[:], psum_b[:, 0 : 2 * NW], AFT.Tanh)
                return sig_if, tanh_g

            def emit_sig_o(d, psum_b):
                sig_o = wpool.tile([128, 2 * NW], DT, name="sig_o", tag=f"sig_o_{d}")
                nc.scalar.activation(sig_o[:], psum_b[:, 2 * NW : 4 * NW], AFT.Sigmoid)
                return sig_o

            def emit_cell(d, t, sig_if, tanh_g):
                # c = sig(f) * c + sig(i) * tanh(g)
                c_prev = c_cur[d]
                c_new = spool.tile([128, 2 * NW], DT, name=f"c_{d}", tag=f"c_{d}")
                if c_prev is None:
                    nc.vector.tensor_mul(c_new[:], sig_if[:, 0 : 2 * NW], tanh_g[:])
                else:
                    m1 = wpool.tile([128, 2 * NW], DT, name="m1", tag=f"m1_{d}")
                    nc.vector.tensor_mul(m1[:], sig_if[:, 2 * NW : 4 * NW], c_prev[:])
                    m2 = wpool.tile([128, 2 * NW], DT, name="m2", tag=f"m2_{d}")
                    nc.vector.tensor_mul(m2[:], sig_if[:, 0 : 2 * NW], tanh_g[:])
                    nc.vector.tensor_add(c_new[:], m1[:], m2[:])
                c_cur[d] = c_new

            def emit_tanh_c(d):
                tanh_c = wpool.tile([128, 2 * NW], DT, name="tanh_c", tag=f"tanh_c_{d}")
                nc.scalar.activation(tanh_c[:], c_cur[d][:], AFT.Tanh)
                return tanh_c

            def emit_h(d, t, sig_o, tanh_c):
                # h = sig(o) * tanh(c)
                if t == L - 1:
                    off = 0 if d == "f" else 2 * NW
                    nc.vector.tensor_mul(
                        out_sb[:, off : off + 2 * NW], sig_o[:], tanh_c[:]
                    )
                else:
                    h_new = spool.tile([128, 2 * NW], DT, name=f"h_{d}", tag=f"h_{d}")
                    nc.vector.tensor_mul(h_new[:], sig_o[:], tanh_c[:])
                    h_cur[d] = h_new

            for t in range(L):
                pa_f, pb_f = emit_mms("f", t)
                pa_b, pb_b = emit_mms("b", t)
                sig_if_f, tanh_g_f = emit_gates_ifg("f", pa_f, pb_f)
                emit_cell("f", t, sig_if_f, tanh_g_f)
                sig_o_f = emit_sig_o("f", pb_f)
                sig_if_b, tanh_g_b = emit_gates_ifg("b", pa_b, pb_b)
                tc_f = emit_tanh_c("f")
                emit_cell("b", t, sig_if_b, tanh_g_b)
                sig_o_b = emit_sig_o("b", pb_b)
                emit_h("f", t, sig_o_f, tc_f)
                tc_b = emit_tanh_c("b")
                emit_h("b", t, sig_o_b, tc_b)

            nc.sync.dma_start(hout_d[:, 0 : 2 * NW], out_sb[:, 0 : 2 * NW])
            nc.sync.dma_start(hout_d[:, 2 * NW : 4 * NW], out_sb[:, 2 * NW : 4 * NW])

    nc.compile()
    return nc


_NC_CACHE = None


def _get_nc():
    global _NC_CACHE
    if _NC_CACHE is None:
        _NC_CACHE = build_nc()
    return _NC_CACHE


# gate permutation: torch order (i,f,g,o) -> device order (i,f,o,g)
_PERM = np.concatenate([np.arange(0, 512), np.arange(768, 1024), np.arange(512, 768)])


def _np_dt(dt):
    return mybir.dt.np(dt)


def prepare_in_maps(x, embed_table, w_ih_f, w_hh_f, b_ih_f, b_hh_f,
                    w_ih_b, w_hh_b, b_ih_b, b_hh_b):
    cdt = _np_dt(DT)
    ids = np.asarray(x).reshape(B * T, L).astype(np.int64)

    shared = {}
    for d, w_ih, w_hh, b_ih, b_hh in (
        ("f", w_ih_f, w_hh_f, b_ih_f, b_hh_f),
        ("b", w_ih_b, w_hh_b, b_ih_b, b_hh_b),
    ):
        w_ih = np.asarray(w_ih, np.float32)[_PERM]
        w_hh = np.asarray(w_hh, np.float32)[_PERM]
        b = (np.asarray(b_ih, np.float32) + np.asarray(b_hh, np.float32))[_PERM]
        fused = np.asarray(embed_table, np.float32) @ w_ih.T + b[None, :]
        shared[f"fused_{d}"] = np.ascontiguousarray(fused.astype(cdt))
        shared[f"whh_{d}"] = np.ascontiguousarray(
            w_hh.T.reshape(2, 128, G4).astype(cdt)
        )

    vrange = np.arange(VOCAB)
    in_maps = []
    for c in range(N_CORES):
        ids_c = ids[c * NW : (c + 1) * NW]  # [NW, L]
        oh = (ids_c.T[:, None, :] == vrange[None, :, None]).astype(cdt)  # [L,V,NW]
        m = dict(shared)
        m["oh"] = np.ascontiguousarray(oh)
        in_maps.append(m)
    return in_maps


def assemble_output(results):
    ys = []
    for c in range(N_CORES):
        hout = results[c]["hout"].astype(np.float32)  # [128, 4*NW]
        hf = np.concatenate([hout[:, 0:NW], hout[:, NW : 2 * NW]], axis=0)  # [H,NW]
        hb = np.concatenate([hout[:, 2 * NW : 3 * NW], hout[:, 3 * NW : 4 * NW]], axis=0)
        ys.append(np.concatenate([hf.T, hb.T], axis=1))  # [NW, 2H]
    y = np.concatenate(ys, axis=0)  # [B*T, 2H]
    return y.reshape(B, T, 2 * H)


def run(in_maps, trace=False):
    nc = _get_nc()
    res = run_bass_kernel_spmd(nc, in_maps, core_ids=list(range(N_CORES)), trace=trace)
    return res


def kernel(**inputs) -> np.ndarray:
    in_maps = prepare_in_maps(**inputs)
    res = run(in_maps, trace=False)
    return assemble_output(res.results)


# revision 15
# speedup vs baseline: 1.2031x; 1.0021x over previous
"""Char-level BiLSTM embedder on 8 NeuronCores (Trainium2, Bass/Tile).

Computation: x[B=32,T=128,L=16] char ids -> embed[E=512] -> fwd+bwd LSTM(H=256)
over the L=16 chars of each of the N=B*T=4096 independent words -> final hidden
states concatenated -> y[B,T,2H=512].

Strategy:
  - Data parallel over N: 512 words per core.
  - Embedding lookup + input projection + bias fused on HOST into a single
    [V=128, 4H] LUT per direction:  fused[v,:] = embed[v] @ w_ih.T + b.
    On device the per-step input contribution is a K=128 matmul with a
    one-hot rhs (built on host), accumulated into the same PSUM group as
    the recurrent h matmuls.
  - Everything device-side is feature-major: gates/c/h live as
    [feature-chunk on partitions, words on free dim], so h feeds the next
    step's matmul rhs directly and no transposes are ever needed.
  - Gate order permuted to (i,f,o,g) so activations batch into 3 big ACT ops.
  - fwd and bwd directions interleave per step to hide recurrence latency.
  - Host does the final [2H,n] -> [n,2H] transpose and core concat.
"""

import sys

sys.path.insert(0, "/opt/trn_rl_repo")

import numpy as np
import concourse.bass as bass
import concourse.bacc as bacc
import concourse.mybir as mybir
import concourse.tile as tile
from concourse.bass_utils import run_bass_kernel_spmd
from concourse.tile_rust import add_dep_helper

# problem constants (hardcoded per harness contract)
B, T, L = 32, 128, 16
VOCAB, E, H = 128, 512, 256
G4 = 4 * H  # 1024
N_CORES = 8
NW = (B * T) // N_CORES  # 512 words per core

F32 = mybir.dt.float32
# compute dtype for matmul operands / gate activations. bf16 halves PE time
# (fp32 matmuls decompose into 2 passes) and enables DVE 2x modes; the cell
# state c and all PSUM accumulation stay fp32.
DT = mybir.dt.bfloat16

AFT = mybir.ActivationFunctionType


def build_nc():
    nc = bacc.Bacc()

    # onehots: t=0 and t=15 land first (tiny DMAs) so both directions can
    # start immediately; the middle steps come as two bulk DMAs.
    oh_d = nc.dram_tensor("oh", [L, VOCAB, NW], DT, kind="ExternalInput")
    fused_dd = {
        d: nc.dram_tensor(f"fused_{d}", [VOCAB, G4], DT, kind="ExternalInput")
        for d in "fb"
    }
    whh_dd = {
        d: nc.dram_tensor(f"whh_{d}", [2, 128, G4], DT, kind="ExternalInput")
        for d in "fb"
    }
    hout_d = nc.dram_tensor("hout", [128, 4 * NW], DT, kind="ExternalOutput")

    with tile.TileContext(nc) as tc:
        with (
            tc.tile_pool(name="const", bufs=1) as cpool,
            tc.tile_pool(name="work", bufs=2) as wpool,
            tc.tile_pool(name="state", bufs=2) as spool,
            tc.tile_pool(name="psum", bufs=2, space=bass.MemorySpace.PSUM) as ppool,
        ):
            # --- load constants -------------------------------------------
            fused = {}
            whh = {}
            oh_ends = {}
            for d, te in (("f", 0), ("b", L - 1)):
                fu = cpool.tile([128, G4], DT, name=f"fused_{d}_sb", tag=f"fused_{d}")
                nc.sync.dma_start(fu[:], fused_dd[d][:])
                fused[d] = fu
                ot = cpool.tile([128, NW], DT, name=f"oh_e{te}", tag=f"oh_e{te}")
                nc.sync.dma_start(ot[:], oh_d[te])
                oh_ends[te] = ot
            for d in "fb":
                w = cpool.tile([128, 2 * G4], DT, name=f"whh_{d}_sb", tag=f"whh_{d}")
                nc.sync.dma_start(
                    w[:].rearrange("p (k g) -> p k g", k=2),
                    whh_dd[d].rearrange("k p g -> p k g"),
                )
                whh[d] = [w[:, 0:G4], w[:, G4 : 2 * G4]]
            oh_mid = {}
            for lo, hi in ((1, 8), (8, 15)):
                om = cpool.tile([128, 7 * NW], DT, name=f"oh_m{lo}", tag=f"oh_m{lo}")
                nc.sync.dma_start(
                    om[:].rearrange("p (t n) -> p t n", t=7),
                    oh_d[lo:hi].rearrange("t p n -> p t n"),
                )
                oh_mid[lo] = om

            def oh_rhs(t):
                if t in oh_ends:
                    return oh_ends[t][:]
                lo = 1 if t < 8 else 8
                return oh_mid[lo][:, (t - lo) * NW : (t - lo + 1) * NW]

            out_sb = cpool.tile([128, 4 * NW], DT, name="out_sb", tag="out_sb")

            c_cur = {"f": None, "b": None}
            h_cur = {"f": None, "b": None}

            # --- recurrent steps ------------------------------------------
            # psum_a chunks: i0,i1,f0,f1 (all sigmoid)
            # psum_b chunks: g0,g1,o0,o1 (tanh first so the cell-update
            #   chain can start while the o matmuls still run)
            B_GC = (6, 7, 4, 5)  # psum_b slice jj -> global gate chunk

            def emit_mms(d, t):
                tchar = t if d == "f" else L - 1 - t
                rhs_oh = oh_rhs(tchar)
                h_prev = h_cur[d]
                psum_a = ppool.tile([128, 4 * NW], F32, name="psum_a", tag="ps")
                psum_b = ppool.tile([128, 4 * NW], F32, name="psum_b", tag="ps")
                # all LUT (one-hot) matmuls first: they depend only on
                # constants, so the PE can run them while h is still being
                # computed; the h matmuls follow.
                for ps, gcs in ((psum_a, (0, 1, 2, 3)), (psum_b, B_GC)):
                    for jj, gc in enumerate(gcs):
                        sl = ps[:, jj * NW : (jj + 1) * NW]
                        lhs_f = fused[d][:, gc * 128 : (gc + 1) * 128]
                        nc.tensor.matmul(
                            sl, lhs_f, rhs_oh, start=True, stop=h_prev is None
                        )
                if h_prev is not None:
                    for ps, gcs in ((psum_a, (0, 1, 2, 3)), (psum_b, B_GC)):
                        for jj, gc in enumerate(gcs):
                            sl = ps[:, jj * NW : (jj + 1) * NW]
                            for k in range(2):
                                lhs_h = whh[d][k][:, gc * 128 : (gc + 1) * 128]
                                rhs_h = h_prev[:, k * NW : (k + 1) * NW]
                                nc.tensor.matmul(
                                    sl, lhs_h, rhs_h, start=False, stop=(k == 1)
                                )
                return psum_a, psum_b

            def emit_gates_ifg(d, psum_a, psum_b):
                sig_if = wpool.tile([128, 4 * NW], DT, name="sig_if", tag=f"sig_if_{d}")
                nc.scalar.activation(sig_if[:], psum_a[:], AFT.Sigmoid)
                tanh_g = wpool.tile([128, 2 * NW], DT, name="tanh_g", tag=f"tanh_g_{d}")
                i_tg = nc.scalar.activation(tanh_g[:], psum_b[:, 0 : 2 * NW], AFT.Tanh)
                return sig_if, tanh_g, i_tg

            def emit_sig_o(d, psum_b):
                sig_o = wpool.tile([128, 2 * NW], DT, name="sig_o", tag=f"sig_o_{d}")
                nc.scalar.activation(sig_o[:], psum_b[:, 2 * NW : 4 * NW], AFT.Sigmoid)
                return sig_o

            def emit_cell(d, t, sig_if, tanh_g):
                # c = sig(f) * c + sig(i) * tanh(g)
                c_prev = c_cur[d]
                c_new = spool.tile([128, 2 * NW], DT, name=f"c_{d}", tag=f"c_{d}")
                if c_prev is None:
                    nc.vector.tensor_mul(c_new[:], sig_if[:, 0 : 2 * NW], tanh_g[:])
                else:
                    m1 = wpool.tile([128, 2 * NW], DT, name="m1", tag=f"m1_{d}")
                    nc.vector.tensor_mul(m1[:], sig_if[:, 2 * NW : 4 * NW], c_prev[:])
                    m2 = wpool.tile([128, 2 * NW], DT, name="m2", tag=f"m2_{d}")
                    nc.vector.tensor_mul(m2[:], sig_if[:, 0 : 2 * NW], tanh_g[:])
                    nc.vector.tensor_add(c_new[:], m1[:], m2[:])
                c_cur[d] = c_new

            def emit_tanh_c(d, c_tile, after=None):
                tanh_c = wpool.tile([128, 2 * NW], DT, name="tanh_c", tag=f"tanh_c_{d}")
                i = nc.scalar.activation(tanh_c[:], c_tile[:], AFT.Tanh)
                if after is not None:
                    # keep ACT from running this ahead of the other
                    # direction's gate activations (scheduler ordering only)
                    add_dep_helper(after.ins, i.ins, sync=False, reason="act order")
                return tanh_c

            def emit_h(d, t, sig_o, tanh_c):
                # h = sig(o) * tanh(c)
                if t == L - 1:
                    off = 0 if d == "f" else 2 * NW
                    nc.vector.tensor_mul(
                        out_sb[:, off : off + 2 * NW], sig_o[:], tanh_c[:]
                    )
                else:
                    h_new = spool.tile([128, 2 * NW], DT, name=f"h_{d}", tag=f"h_{d}")
                    nc.vector.tensor_mul(h_new[:], sig_o[:], tanh_c[:])
                    h_cur[d] = h_new

            pending_b = None  # (t, sig_o_b, c_tile) awaiting next step's gates
            for t in range(L):
                pa_f, pb_f = emit_mms("f", t)
                sig_if_f, tanh_g_f, i_tg_f = emit_gates_ifg("f", pa_f, pb_f)
                if pending_b is not None:
                    pt, p_sig_o, p_c = pending_b
                    tc_pb = emit_tanh_c("b", p_c, after=i_tg_f)
                    emit_h("b", pt, p_sig_o, tc_pb)
                pa_b, pb_b = emit_mms("b", t)
                emit_cell("f", t, sig_if_f, tanh_g_f)
                sig_o_f = emit_sig_o("f", pb_f)
                sig_if_b, tanh_g_b, i_tg_b = emit_gates_ifg("b", pa_b, pb_b)
                tc_f = emit_tanh_c("f", c_cur["f"], after=i_tg_b)
                emit_cell("b", t, sig_if_b, tanh_g_b)
                sig_o_b = emit_sig_o("b", pb_b)
                emit_h("f", t, sig_o_f, tc_f)
                pending_b = (t, sig_o_b, c_cur["b"])
            pt, p_sig_o, p_c = pending_b
            tc_pb = emit_tanh_c("b", p_c)
            emit_h("b", pt, p_sig_o, tc_pb)

            nc.sync.dma_start(hout_d[:, 0 : 2 * NW], out_sb[:, 0 : 2 * NW])
            nc.sync.dma_start(hout_d[:, 2 * NW : 4 * NW], out_sb[:, 2 * NW : 4 * NW])

    nc.compile()
    return nc


_NC_CACHE = None


def _get_nc():
    global _NC_CACHE
    if _NC_CACHE is None:
        _NC_CACHE = build_nc()
    return _NC_CACHE


# gate permutation: torch order (i,f,g,o) -> device order (i,f,o,g)
_PERM = np.concatenate([np.arange(0, 512), np.arange(768, 1024), np.arange(512, 768)])


def _np_dt(dt):
    return mybir.dt.np(dt)


def prepare_in_maps(x, embed_table, w_ih_f, w_hh_f, b_ih_f, b_hh_f,
                    w_ih_b, w_hh_b, b_ih_b, b_hh_b):
    cdt = _np_dt(DT)
    ids = np.asarray(x).reshape(B * T, L).astype(np.int64)

    shared = {}
    for d, w_ih, w_hh, b_ih, b_hh in (
        ("f", w_ih_f, w_hh_f, b_ih_f, b_hh_f),
        ("b", w_ih_b, w_hh_b, b_ih_b, b_hh_b),
    ):
        w_ih = np.asarray(w_ih, np.float32)[_PERM]
        w_hh = np.asarray(w_hh, np.float32)[_PERM]
        b = (np.asarray(b_ih, np.float32) + np.asarray(b_hh, np.float32))[_PERM]
        fused = np.asarray(embed_table, np.float32) @ w_ih.T + b[None, :]
        shared[f"fused_{d}"] = np.ascontiguousarray(fused.astype(cdt))
        shared[f"whh_{d}"] = np.ascontiguousarray(
            w_hh.T.reshape(2, 128, G4).astype(cdt)
        )

    vrange = np.arange(VOCAB)
    in_maps = []
    for c in range(N_CORES):
        ids_c = ids[c * NW : (c + 1) * NW]  # [NW, L]
        oh = (ids_c.T[:, None, :] == vrange[None, :, None]).astype(cdt)  # [L,V,NW]
        m = dict(shared)
        m["oh"] = np.ascontiguousarray(oh)
        in_maps.append(m)
    return in_maps


def assemble_output(results):
    ys = []
    for c in range(N_CORES):
        hout = results[c]["hout"].astype(np.float32)  # [128, 4*NW]
        hf = np.concatenate([hout[:, 0:NW], hout[:, NW : 2 * NW]], axis=0)  # [H,NW]
        hb = np.concatenate([hout[:, 2 * NW : 3 * NW], hout[:, 3 * NW : 4 * NW]], axis=0)
        ys.append(np.concatenate([hf.T, hb.T], axis=1))  # [NW, 2H]
    y = np.concatenate(ys, axis=0)  # [B*T, 2H]
    return y.reshape(B, T, 2 * H)


def run(in_maps, trace=False):
    nc = _get_nc()
    res = run_bass_kernel_spmd(nc, in_maps, core_ids=list(range(N_CORES)), trace=trace)
    return res


def kernel(**inputs) -> np.ndarray:
    in_maps = prepare_in_maps(**inputs)
    res = run(in_maps, trace=False)
    return assemble_output(res.results)


# revision 16
# speedup vs baseline: 1.2127x; 1.0080x over previous
"""Char-level BiLSTM embedder on 8 NeuronCores (Trainium2, Bass/Tile).

Computation: x[B=32,T=128,L=16] char ids -> embed[E=512] -> fwd+bwd LSTM(H=256)
over the L=16 chars of each of the N=B*T=4096 independent words -> final hidden
states concatenated -> y[B,T,2H=512].

Strategy:
  - Data parallel over N: 512 words per core.
  - Embedding lookup + input projection + bias fused on HOST into a single
    [V=128, 4H] LUT per direction:  fused[v,:] = embed[v] @ w_ih.T + b.
    On device the per-step input contribution is a K=128 matmul with a
    one-hot rhs (built on host), accumulated into the same PSUM group as
    the recurrent h matmuls.
  - Everything device-side is feature-major: gates/c/h live as
    [feature-chunk on partitions, words on free dim], so h feeds the next
    step's matmul rhs directly and no transposes are ever needed.
  - Gate order permuted to (i,f,o,g) so activations batch into 3 big ACT ops.
  - fwd and bwd directions interleave per step to hide recurrence latency.
  - Host does the final [2H,n] -> [n,2H] transpose and core concat.
"""

import sys

sys.path.insert(0, "/opt/trn_rl_repo")

import numpy as np
import concourse.bass as bass
import concourse.bacc as bacc
import concourse.mybir as mybir
import concourse.tile as tile
from concourse.bass_utils import run_bass_kernel_spmd
from concourse.tile_rust import add_dep_helper

# problem constants (hardcoded per harness contract)
B, T, L = 32, 128, 16
VOCAB, E, H = 128, 512, 256
G4 = 4 * H  # 1024
N_CORES = 8
NW = (B * T) // N_CORES  # 512 words per core

F32 = mybir.dt.float32
# compute dtype for matmul operands / gate activations. bf16 halves PE time
# (fp32 matmuls decompose into 2 passes) and enables DVE 2x modes; the cell
# state c and all PSUM accumulation stay fp32.
DT = mybir.dt.bfloat16

AFT = mybir.ActivationFunctionType


def build_nc():
    nc = bacc.Bacc()

    # onehots: t=0 and t=15 land first (tiny DMAs) so both directions can
    # start immediately; the middle steps come as two bulk DMAs.
    oh_d = nc.dram_tensor("oh", [L, VOCAB, NW], DT, kind="ExternalInput")
    fused_dd = {
        d: nc.dram_tensor(f"fused_{d}", [VOCAB, G4], DT, kind="ExternalInput")
        for d in "fb"
    }
    whh_dd = {
        d: nc.dram_tensor(f"whh_{d}", [2, 128, G4], DT, kind="ExternalInput")
        for d in "fb"
    }
    hout_d = nc.dram_tensor("hout", [128, 4 * NW], DT, kind="ExternalOutput")

    with tile.TileContext(nc) as tc:
        with (
            tc.tile_pool(name="const", bufs=1) as cpool,
            tc.tile_pool(name="work", bufs=2) as wpool,
            tc.tile_pool(name="state", bufs=2) as spool,
            tc.tile_pool(name="psum", bufs=2, space=bass.MemorySpace.PSUM) as ppool,
        ):
            # --- load constants -------------------------------------------
            fused = {}
            whh = {}
            oh_ends = {}
            for d, te in (("f", 0), ("b", L - 1)):
                fu = cpool.tile([128, G4], DT, name=f"fused_{d}_sb", tag=f"fused_{d}")
                nc.sync.dma_start(fu[:], fused_dd[d][:])
                fused[d] = fu
                ot = cpool.tile([128, NW], DT, name=f"oh_e{te}", tag=f"oh_e{te}")
                nc.sync.dma_start(ot[:], oh_d[te])
                oh_ends[te] = ot
            for d in "fb":
                w = cpool.tile([128, 2 * G4], DT, name=f"whh_{d}_sb", tag=f"whh_{d}")
                nc.sync.dma_start(
                    w[:].rearrange("p (k g) -> p k g", k=2),
                    whh_dd[d].rearrange("k p g -> p k g"),
                )
                whh[d] = [w[:, 0:G4], w[:, G4 : 2 * G4]]
            oh_mid = {}
            for lo, hi in ((1, 8), (8, 15)):
                om = cpool.tile([128, 7 * NW], DT, name=f"oh_m{lo}", tag=f"oh_m{lo}")
                nc.sync.dma_start(
                    om[:].rearrange("p (t n) -> p t n", t=7),
                    oh_d[lo:hi].rearrange("t p n -> p t n"),
                )
                oh_mid[lo] = om

            def oh_rhs(t):
                if t in oh_ends:
                    return oh_ends[t][:]
                lo = 1 if t < 8 else 8
                return oh_mid[lo][:, (t - lo) * NW : (t - lo + 1) * NW]

            out_sb = cpool.tile([128, 4 * NW], DT, name="out_sb", tag="out_sb")

            # HAM warm-up: dummy matmuls on a zeroed tile, issued while the
            # input DMAs are still in flight, so the PE clock gate reaches
            # 2.4 GHz before the first real matmul. Results are overwritten
            # by the first real accumulation group (start=True).
            warm_src = wpool.tile([128, NW], DT, name="warm_src", tag="warm_src", bufs=1)
            nc.gpsimd.memset(warm_src[:], 0.0)
            warm_ps = ppool.tile([128, 4 * NW], F32, name="warm_ps", tag="ps")
            for wj in range(12):
                nc.tensor.matmul(
                    warm_ps[:, (wj % 4) * NW : (wj % 4 + 1) * NW],
                    warm_src[:, 0:128],
                    warm_src[:],
                    start=True,
                    stop=True,
                )

            c_cur = {"f": None, "b": None}
            h_cur = {"f": None, "b": None}

            # --- recurrent steps ------------------------------------------
            # psum_a chunks: i0,i1,f0,f1 (all sigmoid)
            # psum_b chunks: g0,g1,o0,o1 (tanh first so the cell-update
            #   chain can start while the o matmuls still run)
            B_GC = (6, 7, 4, 5)  # psum_b slice jj -> global gate chunk

            def emit_mms(d, t):
                tchar = t if d == "f" else L - 1 - t
                rhs_oh = oh_rhs(tchar)
                h_prev = h_cur[d]
                psum_a = ppool.tile([128, 4 * NW], F32, name="psum_a", tag="ps")
                psum_b = ppool.tile([128, 4 * NW], F32, name="psum_b", tag="ps")
                # all LUT (one-hot) matmuls first: they depend only on
                # constants, so the PE can run them while h is still being
                # computed; the h matmuls follow.
                for ps, gcs in ((psum_a, (0, 1, 2, 3)), (psum_b, B_GC)):
                    for jj, gc in enumerate(gcs):
                        sl = ps[:, jj * NW : (jj + 1) * NW]
                        lhs_f = fused[d][:, gc * 128 : (gc + 1) * 128]
                        nc.tensor.matmul(
                            sl, lhs_f, rhs_oh, start=True, stop=h_prev is None
                        )
                if h_prev is not None:
                    for ps, gcs in ((psum_a, (0, 1, 2, 3)), (psum_b, B_GC)):
                        for jj, gc in enumerate(gcs):
                            sl = ps[:, jj * NW : (jj + 1) * NW]
                            for k in range(2):
                                lhs_h = whh[d][k][:, gc * 128 : (gc + 1) * 128]
                                rhs_h = h_prev[:, k * NW : (k + 1) * NW]
                                nc.tensor.matmul(
                                    sl, lhs_h, rhs_h, start=False, stop=(k == 1)
                                )
                return psum_a, psum_b

            def emit_gates_ifg(d, psum_a, psum_b):
                sig_if = wpool.tile([128, 4 * NW], DT, name="sig_if", tag=f"sig_if_{d}")
                nc.scalar.activation(sig_if[:], psum_a[:], AFT.Sigmoid)
                tanh_g = wpool.tile([128, 2 * NW], DT, name="tanh_g", tag=f"tanh_g_{d}")
                i_tg = nc.scalar.activation(tanh_g[:], psum_b[:, 0 : 2 * NW], AFT.Tanh)
                return sig_if, tanh_g, i_tg

            def emit_sig_o(d, psum_b):
                sig_o = wpool.tile([128, 2 * NW], DT, name="sig_o", tag=f"sig_o_{d}")
                nc.scalar.activation(sig_o[:], psum_b[:, 2 * NW : 4 * NW], AFT.Sigmoid)
                return sig_o

            def emit_cell(d, t, sig_if, tanh_g):
                # c = sig(f) * c + sig(i) * tanh(g)
                c_prev = c_cur[d]
                c_new = spool.tile([128, 2 * NW], DT, name=f"c_{d}", tag=f"c_{d}")
                if c_prev is None:
                    nc.vector.tensor_mul(c_new[:], sig_if[:, 0 : 2 * NW], tanh_g[:])
                else:
                    m1 = wpool.tile([128, 2 * NW], DT, name="m1", tag=f"m1_{d}")
                    nc.vector.tensor_mul(m1[:], sig_if[:, 2 * NW : 4 * NW], c_prev[:])
                    m2 = wpool.tile([128, 2 * NW], DT, name="m2", tag=f"m2_{d}")
                    nc.vector.tensor_mul(m2[:], sig_if[:, 0 : 2 * NW], tanh_g[:])
                    nc.vector.tensor_add(c_new[:], m1[:], m2[:])
                c_cur[d] = c_new

            def emit_tanh_c(d, c_tile, after=None):
                tanh_c = wpool.tile([128, 2 * NW], DT, name="tanh_c", tag=f"tanh_c_{d}")
                i = nc.scalar.activation(tanh_c[:], c_tile[:], AFT.Tanh)
                if after is not None:
                    # keep ACT from running this ahead of the other
                    # direction's gate activations (scheduler ordering only)
                    add_dep_helper(after.ins, i.ins, sync=False, reason="act order")
                return tanh_c

            def emit_h(d, t, sig_o, tanh_c):
                # h = sig(o) * tanh(c)
                if t == L - 1:
                    off = 0 if d == "f" else 2 * NW
                    nc.vector.tensor_mul(
                        out_sb[:, off : off + 2 * NW], sig_o[:], tanh_c[:]
                    )
                else:
                    h_new = spool.tile([128, 2 * NW], DT, name=f"h_{d}", tag=f"h_{d}")
                    nc.vector.tensor_mul(h_new[:], sig_o[:], tanh_c[:])
                    h_cur[d] = h_new

            pending_b = None  # (t, sig_o_b, c_tile) awaiting next step's gates
            for t in range(L):
                pa_f, pb_f = emit_mms("f", t)
                sig_if_f, tanh_g_f, i_tg_f = emit_gates_ifg("f", pa_f, pb_f)
                if pending_b is not None:
                    pt, p_sig_o, p_c = pending_b
                    tc_pb = emit_tanh_c("b", p_c, after=i_tg_f)
                    emit_h("b", pt, p_sig_o, tc_pb)
                pa_b, pb_b = emit_mms("b", t)
                emit_cell("f", t, sig_if_f, tanh_g_f)
                sig_o_f = emit_sig_o("f", pb_f)
                sig_if_b, tanh_g_b, i_tg_b = emit_gates_ifg("b", pa_b, pb_b)
                tc_f = emit_tanh_c("f", c_cur["f"], after=i_tg_b)
                emit_cell("b", t, sig_if_b, tanh_g_b)
                sig_o_b = emit_sig_o("b", pb_b)
                emit_h("f", t, sig_o_f, tc_f)
                pending_b = (t, sig_o_b, c_cur["b"])
            pt, p_sig_o, p_c = pending_b
            tc_pb = emit_tanh_c("b", p_c)
            emit_h("b", pt, p_sig_o, tc_pb)

            nc.sync.dma_start(hout_d[:, 0 : 2 * NW], out_sb[:, 0 : 2 * NW])
            nc.sync.dma_start(hout_d[:, 2 * NW : 4 * NW], out_sb[:, 2 * NW : 4 * NW])

    nc.compile()
    return nc


_NC_CACHE = None


def _get_nc():
    global _NC_CACHE
    if _NC_CACHE is None:
        _NC_CACHE = build_nc()
    return _NC_CACHE


# gate permutation: torch order (i,f,g,o) -> device order (i,f,o,g)
_PERM = np.concatenate([np.arange(0, 512), np.arange(768, 1024), np.arange(512, 768)])


def _np_dt(dt):
    return mybir.dt.np(dt)


def prepare_in_maps(x, embed_table, w_ih_f, w_hh_f, b_ih_f, b_hh_f,
                    w_ih_b, w_hh_b, b_ih_b, b_hh_b):
    cdt = _np_dt(DT)
    ids = np.asarray(x).reshape(B * T, L).astype(np.int64)

    shared = {}
    for d, w_ih, w_hh, b_ih, b_hh in (
        ("f", w_ih_f, w_hh_f, b_ih_f, b_hh_f),
        ("b", w_ih_b, w_hh_b, b_ih_b, b_hh_b),
    ):
        w_ih = np.asarray(w_ih, np.float32)[_PERM]
        w_hh = np.asarray(w_hh, np.float32)[_PERM]
        b = (np.asarray(b_ih, np.float32) + np.asarray(b_hh, np.float32))[_PERM]
        fused = np.asarray(embed_table, np.float32) @ w_ih.T + b[None, :]
        shared[f"fused_{d}"] = np.ascontiguousarray(fused.astype(cdt))
        shared[f"whh_{d}"] = np.ascontiguousarray(
            w_hh.T.reshape(2, 128, G4).astype(cdt)
        )

    vrange = np.arange(VOCAB)
    in_maps = []
    for c in range(N_CORES):
        ids_c = ids[c * NW : (c + 1) * NW]  # [NW, L]
        oh = (ids_c.T[:, None, :] == vrange[None, :, None]).astype(cdt)  # [L,V,NW]
        m = dict(shared)
        m["oh"] = np.ascontiguousarray(oh)
        in_maps.append(m)
    return in_maps


def assemble_output(results):
    ys = []
    for c in range(N_CORES):
        hout = results[c]["hout"].astype(np.float32)  # [128, 4*NW]
        hf = np.concatenate([hout[:, 0:NW], hout[:, NW : 2 * NW]], axis=0)  # [H,NW]
        hb = np.concatenate([hout[:, 2 * NW : 3 * NW], hout[:, 3 * NW : 4 * NW]], axis=0)
        ys.append(np.concatenate([hf.T, hb.T], axis=1))  # [NW, 2H]
    y = np.concatenate(ys, axis=0)  # [B*T, 2H]
    return y.reshape(B, T, 2 * H)


def run(in_maps, trace=False):
    nc = _get_nc()
    res = run_bass_kernel_spmd(nc, in_maps, core_ids=list(range(N_CORES)), trace=trace)
    return res


def kernel(**inputs) -> np.ndarray:
    in_maps = prepare_in_maps(**inputs)
    res = run(in_maps, trace=False)
    return assemble_output(res.results)


# revision 17
# speedup vs baseline: 1.2207x; 1.0065x over previous
"""Char-level BiLSTM embedder on 8 NeuronCores (Trainium2, Bass/Tile).

Computation: x[B=32,T=128,L=16] char ids -> embed[E=512] -> fwd+bwd LSTM(H=256)
over the L=16 chars of each of the N=B*T=4096 independent words -> final hidden
states concatenated -> y[B,T,2H=512].

Strategy:
  - Data parallel over N: 512 words per core.
  - Embedding lookup + input projection + bias fused on HOST into a single
    [V=128, 4H] LUT per direction:  fused[v,:] = embed[v] @ w_ih.T + b.
    On device the per-step input contribution is a K=128 matmul with a
    one-hot rhs (built on host), accumulated into the same PSUM group as
    the recurrent h matmuls.
  - Everything device-side is feature-major: gates/c/h live as
    [feature-chunk on partitions, words on free dim], so h feeds the next
    step's matmul rhs directly and no transposes are ever needed.
  - Gate order permuted to (i,f,o,g) so activations batch into 3 big ACT ops.
  - fwd and bwd directions interleave per step to hide recurrence latency.
  - Host does the final [2H,n] -> [n,2H] transpose and core concat.
"""

import sys

sys.path.insert(0, "/opt/trn_rl_repo")

import numpy as np
import concourse.bass as bass
import concourse.bacc as bacc
import concourse.mybir as mybir
import concourse.tile as tile
from concourse.bass_utils import run_bass_kernel_spmd
from concourse.tile_rust import add_dep_helper

# problem constants (hardcoded per harness contract)
B, T, L = 32, 128, 16
VOCAB, E, H = 128, 512, 256
G4 = 4 * H  # 1024
N_CORES = 8
NW = (B * T) // N_CORES  # 512 words per core

F32 = mybir.dt.float32
# compute dtype for matmul operands / gate activations. bf16 halves PE time
# (fp32 matmuls decompose into 2 passes) and enables DVE 2x modes; the cell
# state c and all PSUM accumulation stay fp32.
DT = mybir.dt.bfloat16

AFT = mybir.ActivationFunctionType


def build_nc():
    nc = bacc.Bacc()

    # onehots: t=0 and t=15 land first (tiny DMAs) so both directions can
    # start immediately; the middle steps come as two bulk DMAs.
    oh_d = nc.dram_tensor("oh", [L, VOCAB, NW], DT, kind="ExternalInput")
    fused_dd = {
        d: nc.dram_tensor(f"fused_{d}", [VOCAB, G4], DT, kind="ExternalInput")
        for d in "fb"
    }
    whh_dd = {
        d: nc.dram_tensor(f"whh_{d}", [2, 128, G4], DT, kind="ExternalInput")
        for d in "fb"
    }
    hout_d = nc.dram_tensor("hout", [128, 4 * NW], DT, kind="ExternalOutput")

    with tile.TileContext(nc) as tc:
        with (
            tc.tile_pool(name="const", bufs=1) as cpool,
            tc.tile_pool(name="work", bufs=2) as wpool,
            tc.tile_pool(name="state", bufs=2) as spool,
            tc.tile_pool(name="psum", bufs=2, space=bass.MemorySpace.PSUM) as ppool,
        ):
            # --- load constants -------------------------------------------
            fused = {}
            whh = {}
            oh_ends = {}
            for d, te in (("f", 0), ("b", L - 1)):
                fu = cpool.tile([128, G4], DT, name=f"fused_{d}_sb", tag=f"fused_{d}")
                nc.sync.dma_start(fu[:], fused_dd[d][:])
                fused[d] = fu
                ot = cpool.tile([128, NW], DT, name=f"oh_e{te}", tag=f"oh_e{te}")
                nc.sync.dma_start(ot[:], oh_d[te])
                oh_ends[te] = ot
            for d in "fb":
                w = cpool.tile([128, 2 * G4], DT, name=f"whh_{d}_sb", tag=f"whh_{d}")
                nc.sync.dma_start(
                    w[:].rearrange("p (k g) -> p k g", k=2),
                    whh_dd[d].rearrange("k p g -> p k g"),
                )
                whh[d] = [w[:, 0:G4], w[:, G4 : 2 * G4]]
            oh_mid = {}
            for lo, hi in ((1, 8), (8, 15)):
                om = cpool.tile([128, 7 * NW], DT, name=f"oh_m{lo}", tag=f"oh_m{lo}")
                nc.sync.dma_start(
                    om[:].rearrange("p (t n) -> p t n", t=7),
                    oh_d[lo:hi].rearrange("t p n -> p t n"),
                )
                oh_mid[lo] = om

            def oh_rhs(t):
                if t in oh_ends:
                    return oh_ends[t][:]
                lo = 1 if t < 8 else 8
                return oh_mid[lo][:, (t - lo) * NW : (t - lo + 1) * NW]

            out_sb = cpool.tile([128, 4 * NW], DT, name="out_sb", tag="out_sb")

            # HAM warm-up: dummy matmuls on a zeroed tile, issued while the
            # input DMAs are still in flight, so the PE clock gate reaches
            # 2.4 GHz before the first real matmul. Results are overwritten
            # by the first real accumulation group (start=True).
            warm_src = wpool.tile([128, NW], DT, name="warm_src", tag="warm_src", bufs=1)
            nc.gpsimd.memset(warm_src[:], 0.0)
            warm_ps = ppool.tile([128, 4 * NW], F32, name="warm_ps", tag="ps")
            for wj in range(22):
                nc.tensor.matmul(
                    warm_ps[:, (wj % 4) * NW : (wj % 4) * NW + 128],
                    warm_src[:, 0:128],
                    warm_src[:, 0:128],
                    start=True,
                    stop=True,
                )

            c_cur = {"f": None, "b": None}
            h_cur = {"f": None, "b": None}

            # --- recurrent steps ------------------------------------------
            # psum_a chunks: i0,i1,f0,f1 (all sigmoid)
            # psum_b chunks: g0,g1,o0,o1 (tanh first so the cell-update
            #   chain can start while the o matmuls still run)
            B_GC = (6, 7, 4, 5)  # psum_b slice jj -> global gate chunk

            def emit_mms(d, t):
                tchar = t if d == "f" else L - 1 - t
                rhs_oh = oh_rhs(tchar)
                h_prev = h_cur[d]
                psum_a = ppool.tile([128, 4 * NW], F32, name="psum_a", tag="ps")
                psum_b = ppool.tile([128, 4 * NW], F32, name="psum_b", tag="ps")
                # all LUT (one-hot) matmuls first: they depend only on
                # constants, so the PE can run them while h is still being
                # computed; the h matmuls follow.
                for ps, gcs in ((psum_a, (0, 1, 2, 3)), (psum_b, B_GC)):
                    for jj, gc in enumerate(gcs):
                        sl = ps[:, jj * NW : (jj + 1) * NW]
                        lhs_f = fused[d][:, gc * 128 : (gc + 1) * 128]
                        nc.tensor.matmul(
                            sl, lhs_f, rhs_oh, start=True, stop=h_prev is None
                        )
                if h_prev is not None:
                    for ps, gcs in ((psum_a, (0, 1, 2, 3)), (psum_b, B_GC)):
                        for jj, gc in enumerate(gcs):
                            sl = ps[:, jj * NW : (jj + 1) * NW]
                            for k in range(2):
                                lhs_h = whh[d][k][:, gc * 128 : (gc + 1) * 128]
                                rhs_h = h_prev[:, k * NW : (k + 1) * NW]
                                nc.tensor.matmul(
                                    sl, lhs_h, rhs_h, start=False, stop=(k == 1)
                                )
                return psum_a, psum_b

            def emit_gates_ifg(d, psum_a, psum_b):
                sig_if = wpool.tile([128, 4 * NW], DT, name="sig_if", tag=f"sig_if_{d}")
                nc.scalar.activation(sig_if[:], psum_a[:], AFT.Sigmoid)
                tanh_g = wpool.tile([128, 2 * NW], DT, name="tanh_g", tag=f"tanh_g_{d}")
                i_tg = nc.scalar.activation(tanh_g[:], psum_b[:, 0 : 2 * NW], AFT.Tanh)
                return sig_if, tanh_g, i_tg

            def emit_sig_o(d, psum_b):
                sig_o = wpool.tile([128, 2 * NW], DT, name="sig_o", tag=f"sig_o_{d}")
                nc.scalar.activation(sig_o[:], psum_b[:, 2 * NW : 4 * NW], AFT.Sigmoid)
                return sig_o

            def emit_cell(d, t, sig_if, tanh_g):
                # c = sig(f) * c + sig(i) * tanh(g)
                c_prev = c_cur[d]
                c_new = spool.tile([128, 2 * NW], DT, name=f"c_{d}", tag=f"c_{d}")
                if c_prev is None:
                    nc.vector.tensor_mul(c_new[:], sig_if[:, 0 : 2 * NW], tanh_g[:])
                else:
                    m1 = wpool.tile([128, 2 * NW], DT, name="m1", tag=f"m1_{d}")
                    nc.vector.tensor_mul(m1[:], sig_if[:, 2 * NW : 4 * NW], c_prev[:])
                    m2 = wpool.tile([128, 2 * NW], DT, name="m2", tag=f"m2_{d}")
                    nc.vector.tensor_mul(m2[:], sig_if[:, 0 : 2 * NW], tanh_g[:])
                    nc.vector.tensor_add(c_new[:], m1[:], m2[:])
                c_cur[d] = c_new

            def emit_tanh_c(d, c_tile, after=None):
                tanh_c = wpool.tile([128, 2 * NW], DT, name="tanh_c", tag=f"tanh_c_{d}")
                i = nc.scalar.activation(tanh_c[:], c_tile[:], AFT.Tanh)
                if after is not None:
                    # keep ACT from running this ahead of the other
                    # direction's gate activations (scheduler ordering only)
                    add_dep_helper(after.ins, i.ins, sync=False, reason="act order")
                return tanh_c

            def emit_h(d, t, sig_o, tanh_c):
                # h = sig(o) * tanh(c)
                if t == L - 1:
                    off = 0 if d == "f" else 2 * NW
                    nc.vector.tensor_mul(
                        out_sb[:, off : off + 2 * NW], sig_o[:], tanh_c[:]
                    )
                else:
                    h_new = spool.tile([128, 2 * NW], DT, name=f"h_{d}", tag=f"h_{d}")
                    nc.vector.tensor_mul(h_new[:], sig_o[:], tanh_c[:])
                    h_cur[d] = h_new

            pending_b = None  # (t, sig_o_b, c_tile) awaiting next step's gates
            for t in range(L):
                pa_f, pb_f = emit_mms("f", t)
                sig_if_f, tanh_g_f, i_tg_f = emit_gates_ifg("f", pa_f, pb_f)
                if pending_b is not None:
                    pt, p_sig_o, p_c = pending_b
                    tc_pb = emit_tanh_c("b", p_c, after=i_tg_f)
                    emit_h("b", pt, p_sig_o, tc_pb)
                pa_b, pb_b = emit_mms("b", t)
                emit_cell("f", t, sig_if_f, tanh_g_f)
                sig_o_f = emit_sig_o("f", pb_f)
                sig_if_b, tanh_g_b, i_tg_b = emit_gates_ifg("b", pa_b, pb_b)
                tc_f = emit_tanh_c("f", c_cur["f"], after=i_tg_b)
                emit_cell("b", t, sig_if_b, tanh_g_b)
                sig_o_b = emit_sig_o("b", pb_b)
                emit_h("f", t, sig_o_f, tc_f)
                pending_b = (t, sig_o_b, c_cur["b"])
            pt, p_sig_o, p_c = pending_b
            tc_pb = emit_tanh_c("b", p_c)
            emit_h("b", pt, p_sig_o, tc_pb)

            nc.sync.dma_start(hout_d[:, 0 : 2 * NW], out_sb[:, 0 : 2 * NW])
            nc.sync.dma_start(hout_d[:, 2 * NW : 4 * NW], out_sb[:, 2 * NW : 4 * NW])

    nc.compile()
    return nc


_NC_CACHE = None


def _get_nc():
    global _NC_CACHE
    if _NC_CACHE is None:
        _NC_CACHE = build_nc()
    return _NC_CACHE


# gate permutation: torch order (i,f,g,o) -> device order (i,f,o,g)
_PERM = np.concatenate([np.arange(0, 512), np.arange(768, 1024), np.arange(512, 768)])


def _np_dt(dt):
    return mybir.dt.np(dt)


def prepare_in_maps(x, embed_table, w_ih_f, w_hh_f, b_ih_f, b_hh_f,
                    w_ih_b, w_hh_b, b_ih_b, b_hh_b):
    cdt = _np_dt(DT)
    ids = np.asarray(x).reshape(B * T, L).astype(np.int64)

    shared = {}
    for d, w_ih, w_hh, b_ih, b_hh in (
        ("f", w_ih_f, w_hh_f, b_ih_f, b_hh_f),
        ("b", w_ih_b, w_hh_b, b_ih_b, b_hh_b),
    ):
        w_ih = np.asarray(w_ih, np.float32)[_PERM]
        w_hh = np.asarray(w_hh, np.float32)[_PERM]
        b = (np.asarray(b_ih, np.float32) + np.asarray(b_hh, np.float32))[_PERM]
        fused = np.asarray(embed_table, np.float32) @ w_ih.T + b[None, :]
        shared[f"fused_{d}"] = np.ascontiguousarray(fused.astype(cdt))
        shared[f"whh_{d}"] = np.ascontiguousarray(
            w_hh.T.reshape(2, 128, G4).astype(cdt)
        )

    vrange = np.arange(VOCAB)
    in_maps = []
    for c in range(N_CORES):
        ids_c = ids[c * NW : (c + 1) * NW]  # [NW, L]
        oh = (ids_c.T[:, None, :] == vrange[None, :, None]).astype(cdt)  # [L,V,NW]
        m = dict(shared)
        m["oh"] = np.ascontiguousarray(oh)
        in_maps.append(m)
    return in_maps


def assemble_output(results):
    ys = []
    for c in range(N_CORES):
        hout = results[c]["hout"].astype(np.float32)  # [128, 4*NW]
        hf = np.concatenate([hout[:, 0:NW], hout[:, NW : 2 * NW]], axis=0)  # [H,NW]
        hb = np.concatenate([hout[:, 2 * NW : 3 * NW], hout[:, 3 * NW : 4 * NW]], axis=0)
        ys.append(np.concatenate([hf.T, hb.T], axis=1))  # [NW, 2H]
    y = np.concatenate(ys, axis=0)  # [B*T, 2H]
    return y.reshape(B, T, 2 * H)


def run(in_maps, trace=False):
    nc = _get_nc()
    res = run_bass_kernel_spmd(nc, in_maps, core_ids=list(range(N_CORES)), trace=trace)
    return res


def kernel(**inputs) -> np.ndarray:
    in_maps = prepare_in_maps(**inputs)
    res = run(in_maps, trace=False)
    return assemble_output(res.results)
